# revision 15
# baseline (speedup 1.0000x reference)
"""Multi-head causal attention (B=4, T=2048, C=1024, H=16, D=64) on 8 trn2 cores.

Sharding: tensor-parallel over heads within batch core-pairs.
  core c -> batch b = c//2, heads hoff..hoff+7 where hoff = (c%2)*8.

v2 design (all-bf16 dataflow):
  - Projections (Q^T/K^T per head-pair, V per key-chunk with a folded ones
    column) are software-pipelined INTO the attention loop as PE filler work:
    slab s+1's projection matmuls are interleaved between slab s's attention
    groups, because attention is ACT(exp)-bound while projections are pure PE.
  - Scores per head in S^T = [key, query] orientation, exp without
    max-subtraction (scores ~N(0, 0.25^2)), single merged causal
    affine_select per band group.
  - AV in O-orientation: stationary = exp(S^T) chunk [128k, 128q], moving =
    V [128k, 65] (col 64 = ones -> softmax sums land in ov col 64). 65-row
    bf16 matmuls halve AV PE time vs the O^T orientation.
  - Normalization: per-partition reciprocal + tensor_tensor multiply (queries
    are on partitions in O-layout), then O is transposed back to O^T via
    128-row matmuls against an identity for the output projection.
  - Output projection to partial y^T (+bo/2) in bf16; pairwise ReduceScatter
    (bf16 payload, 4 t-slabs) sums partner partials; core even keeps
    c' 0:512, odd keeps 512:1024.
Host reassembles the [B, T, C] f32 output by transposing/concatenating slabs.
"""

import numpy as np
import ml_dtypes

import concourse.bass as bass
import concourse.mybir as mybir
from concourse import bacc
from concourse.tile import TileContext
from concourse.bass_utils import run_bass_kernel_spmd

F32 = mybir.dt.float32
BF16 = mybir.dt.bfloat16

B, T, C = 4, 2048, 1024
H, D = 16, 64
HC = 8           # heads per core
NPAIR = HC // 2  # head pairs
CCn = C // 128   # 8 contraction chunks
TTn = T // 512   # 4 slabs of 512
JCn = T // 128   # 16 key chunks of 128
N_CORES = 8
RG = [[0, 1], [2, 3], [4, 5], [6, 7]]


def build_nc(with_rs: bool = True, debug: bool = False):
    nc = bacc.Bacc(None, target_bir_lowering=False)

    xT = nc.declare_dram_parameter("xT", [C, T], BF16, isOutput=False)
    wq = nc.declare_dram_parameter("wq", [C, 512], BF16, isOutput=False)
    wk = nc.declare_dram_parameter("wk", [C, 512], BF16, isOutput=False)
    wv = nc.declare_dram_parameter("wv", [C, 512], BF16, isOutput=False)
    wot = nc.declare_dram_parameter("wot", [512, C], BF16, isOutput=False)
    bo2 = nc.declare_dram_parameter("bo2", [128, 8], F32, isOutput=False)
    y = nc.declare_dram_parameter("y", [TTn, 512, 512], BF16, isOutput=True)
    if debug:
        dbg_kq = nc.declare_dram_parameter("dbg_kq", [2, 128, 512], BF16,
                                           isOutput=True)
        dbg_v = nc.declare_dram_parameter("dbg_v", [4, 128, 520], BF16,
                                          isOutput=True)
        dbg_pt = nc.declare_dram_parameter("dbg_pt", [2, 128, 1024], BF16,
                                           isOutput=True)
        dbg_ov = nc.declare_dram_parameter("dbg_ov", [128, 260], F32,
                                           isOutput=True)
        dbg_op = nc.declare_dram_parameter("dbg_op", [128, 512], BF16,
                                           isOutput=True)
        dbg_ot = nc.declare_dram_parameter("dbg_ot", [128, 512], BF16,
                                           isOutput=True)

    with TileContext(nc) as tc:
        with (
            tc.tile_pool(name="persist", bufs=1) as sb,
            tc.tile_pool(name="psum", bufs=1, space="PSUM") as psum,
            tc.tile_pool(name="dram", bufs=1, space="DRAM") as dram,
        ):
            # ---- persistent SBUF tiles (per-slab splits avoid false deps
            # between interleaved projection writes and attention reads) ----
            qt = {(p, s): sb.tile([128, 512], BF16, tag=f"qt{p}_{s}", name=f"qt{p}_{s}")
                  for p in range(NPAIR) for s in range(TTn)}
            kt = {(p, s): sb.tile([128, 512], BF16, tag=f"kt{p}_{s}", name=f"kt{p}_{s}")
                  for p in range(NPAIR) for s in range(TTn)}
            v = [sb.tile([128, 65 * HC], BF16, tag=f"v{j}", name=f"v{j}") for j in range(JCn)]
            ot = {(p, s): sb.tile([128, 512], BF16, tag=f"ot{p}_{s}", name=f"ot{p}_{s}")
                  for p in range(NPAIR) for s in range(TTn)}
            wqb = sb.tile([128, CCn * 512], BF16, tag="wqb", name="wqb")
            wkb = sb.tile([128, CCn * 512], BF16, tag="wkb", name="wkb")
            wvb = sb.tile([128, CCn * 512], BF16, tag="wvb", name="wvb")
            wob = sb.tile([128, 4 * C], BF16, tag="wob", name="wob")
            ones8 = sb.tile([128, HC], BF16, tag="ones8")
            ident = sb.tile([128, 128], BF16, tag="ident")
            bo_sb = sb.tile([128, 8], F32, tag="bo_sb")

            y_part = dram.tile([TTn, 1024, 512], BF16)
            rs_out = dram.tile([TTn, 512, 512], BF16)

            # ---- prologue DMAs: one batched strided transfer per tensor
            # (per-DMA queue overhead ~0.6us dominates small transfers) ----
            xtb_of = {}

            def issue_xts(s):
                i0 = s * 512
                t = sb.tile([128, CCn * 512], BF16, tag="xtb", bufs=2,
                            name=f"xtb{s}")
                nc.sync.dma_start(
                    out=t[:].rearrange("p (cc t) -> p cc t", cc=CCn),
                    in_=xT[:, i0:i0 + 512].rearrange(
                        "(cc p) t -> p cc t", cc=CCn))
                xtb_of[s] = t

            nc.sync.dma_start(
                out=wkb[:].rearrange("p (cc j) -> p cc j", cc=CCn),
                in_=wk[:].rearrange("(cc p) j -> p cc j", cc=CCn))
            issue_xts(0)
            nc.sync.dma_start(
                out=wqb[:].rearrange("p (cc j) -> p cc j", cc=CCn),
                in_=wq[:].rearrange("(cc p) j -> p cc j", cc=CCn))
            nc.sync.dma_start(
                out=wvb[:].rearrange("p (cc j) -> p cc j", cc=CCn),
                in_=wv[:].rearrange("(cc p) j -> p cc j", cc=CCn))
            nc.sync.dma_start(
                out=wob[:].rearrange("p (cl j) -> p cl j", cl=4),
                in_=wot[:].rearrange("(cl p) j -> p cl j", cl=4))
            nc.sync.dma_start(out=bo_sb[:], in_=bo2[:])
            nc.vector.memset(ones8[:], 1.0)
            # identity for O -> O^T transposes: memset 1, keep only the diagonal
            nc.vector.memset(ident[:], 1.0)
            nc.gpsimd.affine_select(
                out=ident[:], in_=ident[:], compare_op=mybir.AluOpType.is_ge,
                fill=0.0, base=0, pattern=[[1, 128]], channel_multiplier=-1)
            nc.gpsimd.affine_select(
                out=ident[:], in_=ident[:], compare_op=mybir.AluOpType.is_ge,
                fill=0.0, base=0, pattern=[[-1, 128]], channel_multiplier=1)

            # ---- projection emission units (filler work for the PE) ----
            # Each unit emits ~2-4 matmuls (~0.4us of PE). A group's PSUM tile
            # is allocated by its first unit; the last unit emits the copy out.
            def make_proj_units(s, defer_qk_pairs=()):
                units = []
                deferred = []  # (unit, deadline_slot) for the NEXT slab
                xtb = xtb_of[s]

                def qk_group(wb, pair, dst_tile, defer_dl=None):
                    cell = {}
                    for cc0 in range(0, CCn, 2):
                        def u(cc0=cc0, wb=wb, pair=pair, cell=cell):
                            if cc0 == 0:
                                cell["ps"] = psum.tile([128, 512], F32,
                                                       tag="yps", bufs=2,
                                                       name="qkps")
                            ps = cell["ps"]
                            for cc in (cc0, cc0 + 1):
                                nc.tensor.matmul(
                                    ps[:],
                                    wb[:, cc * 512 + pair * 128:
                                       cc * 512 + pair * 128 + 128],
                                    xtb[:, cc * 512:(cc + 1) * 512],
                                    start=(cc == 0), stop=(cc == CCn - 1),
                                    skip_group_check=True)
                            if cc0 == CCn - 2:
                                nc.vector.tensor_copy(dst_tile[:], ps[:])
                        if defer_dl is None:
                            units.append(u)
                        else:
                            deferred.append((u, defer_dl))

                for p in range(NPAIR):
                    if p in defer_qk_pairs:
                        # needed by next slab: Q at (h=2p, g0), K at (h=2p, g6)
                        qk_group(wkb, p, kt[(p, s)], defer_dl=16 * p + 5)
                        qk_group(wqb, p, qt[(p, s)], defer_dl=16 * p - 1)
                    else:
                        qk_group(wkb, p, kt[(p, s)])
                        qk_group(wqb, p, qt[(p, s)])

                def v_group(jc):
                    jl = jc * 128 - s * 512
                    cell = {}
                    for i in range(4):
                        def u(i=i, jc=jc, jl=jl, cell=cell):
                            if i == 0:
                                cell["ps"] = psum.tile([128, 512], F32,
                                                       tag="yps", bufs=2, name="vps")
                            ps = cell["ps"]
                            for m in range(4):
                                gm = i * 4 + m
                                g, cc = gm // 8, gm % 8
                                nc.tensor.matmul(
                                    ps[:, g * 256:(g + 1) * 256],
                                    xtb[:, cc * 512 + jl:cc * 512 + jl + 128],
                                    wvb[:, cc * 512 + g * 256:
                                        cc * 512 + g * 256 + 256],
                                    start=(cc == 0), stop=(cc == CCn - 1),
                                    skip_group_check=True)
                            if i == 3:
                                vv = v[jc][:].rearrange(
                                    "p (h e) -> p h e", h=HC, e=65)
                                nc.vector.tensor_copy(vv[:, :, 0:64], ps[:])
                                nc.vector.tensor_copy(vv[:, :, 64:65],
                                                      ones8[:])
                        units.append(u)

                for jc in range(4 * s, 4 * s + 4):
                    v_group(jc)
                deferred.sort(key=lambda t: t[1])
                return units, deferred

            # interleave order inside PROJ(0) so tt0/h0 attention can start
            # as early as possible: K0,Q0,V0,V1 then the rest
            units0, _ = make_proj_units(0)
            # units0 layout: [K0(4), Q0(4), K1(4), Q1(4), ... V groups(4x4)]
            order0 = (units0[0:8] + units0[32:40] + units0[8:16]
                      + units0[40:48] + units0[16:32])
            for u in order0:
                u()
            deferred_next = []

            # ---- attention + interleaved projections ----
            pt_pool = sb
            held = None          # pending AV emission for the previous group
            pending_norm = None  # (tt, h, ov) awaiting recip+TT
            opair_cell = {}      # pair -> o_pair staging tile

            def emit_avs(hd):
                tt_, h_, pt_, a_of = hd
                ovt = ov_of[(tt_, h_)]
                for qq in range(4):
                    for idx, (jc, a) in enumerate(a_of):
                        if a <= qq * 128:
                            c0 = idx * 512 + qq * 128
                            # start only once per head: start=True arms a
                            # pending-zero over the whole 2KB PSUM zero
                            # region, so later qq slices first-touch-replace
                            # rather than re-arm (which would wipe earlier
                            # slices' partials on their next accumulate).
                            nc.tensor.matmul(
                                ovt[:, qq * 65:qq * 65 + 65],
                                pt_[:, c0:c0 + 128],
                                v[jc][:, h_ * 65:(h_ + 1) * 65],
                                start=(jc == 0 and qq == 0),
                                stop=(jc == 4 * tt_ + qq),
                                skip_group_check=True)

            ov_of = {}

            def emit_norm(tt, h, ov):
                p, e = h // 2, h % 2
                if e == 0:
                    opair_cell[p] = sb.tile([128, 512], BF16, tag="opair",
                                            bufs=2, name=f"op{tt}{p}")
                opair = opair_cell[p]
                ovr = ov[:].rearrange("p (q o e) -> p q o e", q=4, o=1, e=65)
                rl = sb.tile([128, 4], F32, tag="rl", bufs=2, name="rl")
                nc.vector.reciprocal(
                    rl[:].rearrange("p (q o) -> p q o", q=4, o=1),
                    ov[:].rearrange("p (q e) -> p q e", q=4, e=65)[:, :, 64:65])
                opr = opair[:].rearrange("p (q hh e) -> p q hh e",
                                         q=4, hh=2, e=64)
                nc.vector.tensor_mul(
                    opr[:, :, e:e + 1, :], ovr[:, :, :, 0:64],
                    rl[:].rearrange("p (q o u) -> p q o u", q=4, o=1, u=1)
                    .broadcast_to((128, 4, 1, 64)))
                if debug and (tt, h) == (0, 0):
                    ov_stage = sb.tile([128, 260], F32, tag="dbgov",
                                       name="dbgov")
                    nc.vector.tensor_copy(ov_stage[:], ov[:])
                    nc.sync.dma_start(out=dbg_ov[:], in_=ov_stage[:])
                if e == 1:
                    # pair complete: transpose O -> O^T into ot[(p, tt)]
                    trp = psum.tile([128, 512], F32, tag="yps", bufs=2,
                                    name=f"tr{tt}{p}")
                    for qq in range(4):
                        nc.tensor.matmul(
                            trp[:, qq * 128:(qq + 1) * 128],
                            opair[:, qq * 128:(qq + 1) * 128],
                            ident[:], start=True, stop=True,
                            skip_group_check=True)
                    nc.vector.tensor_copy(ot[(p, tt)][:], trp[:])
                    if debug and (tt, h) == (0, 1):
                        nc.sync.dma_start(out=dbg_op[:], in_=opair[:])
                        nc.sync.dma_start(out=dbg_ot[:], in_=ot[(p, tt)][:])
                    del opair_cell[p]

            def emit_outproj_group(tt, cp):
                yps = psum.tile([128, 512], F32, tag="yps", bufs=2,
                                name=f"yps{tt}{cp}")
                for cl in range(4):
                    nc.tensor.matmul(
                        yps[:], wot_t[cl][:, cp * 128:(cp + 1) * 128],
                        ot[(cl, tt)][:], start=(cl == 0), stop=(cl == 3),
                        skip_group_check=True)
                ysb = sb.tile([128, 512], BF16, tag="ysb", bufs=4, name="ysb")
                nc.vector.tensor_scalar_add(ysb[:], yps[:],
                                            bo_sb[:, cp:cp + 1])
                nc.sync.dma_start(
                    out=y_part[tt, cp * 128:(cp + 1) * 128, :], in_=ysb[:])

            def emit_rs(tt):
                if with_rs:
                    nc.gpsimd.collective_compute(
                        "ReduceScatter", mybir.AluOpType.add,
                        replica_groups=RG,
                        ins=[y_part[tt]], outs=[rs_out[tt]])
                    nc.sync.dma_start(out=y[tt], in_=rs_out[tt])
                else:
                    nc.sync.dma_start(out=y[tt], in_=y_part[tt, 0:512, :])

            for tt in range(TTn):
                i0 = tt * 512
                n_g = 2 * (tt + 1)
                if tt < TTn - 1:
                    issue_xts(tt + 1)
                    proj_units, deferred_next = make_proj_units(
                        tt + 1, defer_qk_pairs=(1, 2, 3) if tt == 2 else ())
                    deferred = []
                else:
                    proj_units = []
                    deferred = deferred_next
                slot_idx = [0]
                emitted = [0]
                total_slots = 8 * n_g
                total_def = len(deferred)

                def emit_filler():
                    si = slot_idx[0]
                    slot_idx[0] += 1
                    rem_slots = total_slots - si
                    if proj_units and rem_slots > 0:
                        n = -(-len(proj_units) // rem_slots)  # ceil
                        for _ in range(min(n, 8)):
                            if proj_units:
                                proj_units.pop(0)()
                    # deferred units: emit when due (deadline) or to keep
                    # proportional pace across the whole slab
                    target = (si + 1) * total_def // max(total_slots, 1)
                    while deferred and (deferred[0][1] <= si + 1
                                        or emitted[0] < target):
                        deferred.pop(0)[0]()
                        emitted[0] += 1

                for h in range(HC):
                    p, e = h // 2, h % 2
                    ov_of[(tt, h)] = psum.tile([128, 260], F32, tag="ovps",
                                               bufs=2, name=f"ov{tt}{h}")
                    for g in range(n_g):
                        jc0, jc1 = 2 * g, 2 * g + 1
                        a0 = max(0, (jc0 - 4 * tt)) * 128
                        a1 = max(0, (jc1 - 4 * tt)) * 128
                        st = psum.tile([128, 1024], F32, tag="stps", bufs=2,
                                       name=f"st{tt}{h}{g}")
                        for k, (jc, a) in enumerate(((jc0, a0), (jc1, a1))):
                            nc.tensor.matmul(
                                st[:, k * 512 + a:(k + 1) * 512],
                                kt[(p, jc // 4)][e * 64:(e + 1) * 64,
                                                 (jc % 4) * 128:
                                                 (jc % 4) * 128 + 128],
                                qt[(p, tt)][e * 64:(e + 1) * 64, a:512],
                                start=True, stop=True, skip_group_check=True)
                        if held is not None:
                            emit_avs(held)
                            held = None
                        if g == 0 and tt >= 1 and h in (1, 2):
                            for g4 in range(4):
                                emit_outproj_group(tt - 1, 4 * (h - 1) + g4)
                            if h == 2:
                                emit_rs(tt - 1)
                        if g == 1 and pending_norm is not None:
                            emit_norm(*pending_norm)
                            pending_norm = None
                        emit_filler()
                        pt = pt_pool.tile([128, 1024], BF16, tag="pt", bufs=7,
                                          name=f"pt{tt}{h}{g}")
                        if debug and tt == 0 and h == 0:
                            nc.vector.memset(pt[:], 0.0)
                        if jc0 >= 4 * tt:
                            # band group: exact per-chunk exp + causal zeroing
                            # (regions outside [k*512+a, (k+1)*512) are never
                            # read downstream, so they stay unwritten)
                            for k, a in ((0, a0), (1, a1)):
                                lo, hi = k * 512 + a, (k + 1) * 512
                                nc.scalar.activation(
                                    pt[:, lo:hi], st[:, lo:hi],
                                    mybir.ActivationFunctionType.Exp)
                                nc.gpsimd.affine_select(
                                    out=pt[:, lo:hi], in_=pt[:, lo:hi],
                                    compare_op=mybir.AluOpType.is_ge,
                                    fill=0.0, base=0,
                                    pattern=[[1, 512 - a]],
                                    channel_multiplier=-1)
                        else:
                            nc.scalar.activation(
                                pt[:, 0:1024], st[:, 0:1024],
                                mybir.ActivationFunctionType.Exp)
                        if debug and tt == 0 and h == 0:
                            nc.sync.dma_start(out=dbg_pt[g], in_=pt[:])
                            if g == 0:
                                nc.sync.dma_start(out=dbg_kq[0],
                                                  in_=kt[(0, 0)][:])
                                nc.sync.dma_start(out=dbg_kq[1],
                                                  in_=qt[(0, 0)][:])
                                for _j in range(4):
                                    nc.sync.dma_start(out=dbg_v[_j],
                                                      in_=v[_j][:])
                        held = (tt, h, pt, ((jc0, a0), (jc1, a1)))
                    if pending_norm is not None:  # tt0: only 2 groups per head
                        emit_norm(*pending_norm)
                        pending_norm = None
                    pending_norm = (tt, h, ov_of[(tt, h)])
                while proj_units:
                    proj_units.pop(0)()
                while deferred:
                    deferred.pop(0)[0]()
                if tt == TTn - 1:
                    if held is not None:
                        emit_avs(held)
                        held = None
                    if pending_norm is not None:
                        emit_norm(*pending_norm)
                        pending_norm = None
                    for cp in range(8):
                        emit_outproj_group(tt, cp)
                    emit_rs(tt)

    nc.compile()
    return nc


_NC_CACHE = {}


def _get_nc(with_rs: bool = True):
    key = bool(with_rs)
    if key not in _NC_CACHE:
        _NC_CACHE[key] = build_nc(with_rs)
    return _NC_CACHE[key]


def make_in_maps(x, Wq, Wk, Wv, Wo, bo):
    bf16 = ml_dtypes.bfloat16
    x = np.asarray(x, dtype=np.float32)
    Wq = np.asarray(Wq, dtype=np.float32)
    Wk = np.asarray(Wk, dtype=np.float32)
    Wv = np.asarray(Wv, dtype=np.float32)
    Wo = np.asarray(Wo, dtype=np.float32)
    bo = np.asarray(bo, dtype=np.float32)

    scale = np.float32(C) ** np.float32(-0.5)
    in_maps = []
    for c in range(N_CORES):
        b, hoff = c // 2, (c % 2) * HC
        heads = slice(hoff, hoff + HC)
        xT_c = np.ascontiguousarray(x[b].T).astype(bf16)             # [C, T]
        wq_c = np.ascontiguousarray(
            np.concatenate(list(Wq[heads] * scale), axis=1)).astype(bf16)
        wk_c = np.ascontiguousarray(
            np.concatenate(list(Wk[heads]), axis=1)).astype(bf16)
        wv_c = np.ascontiguousarray(
            np.concatenate(list(Wv[heads]), axis=1)).astype(bf16)
        wot_c = np.ascontiguousarray(
            Wo[:, hoff * D:(hoff + HC) * D].T).astype(bf16)          # [512, C]
        bo2_c = np.ascontiguousarray((bo / 2.0).reshape(8, 128).T)   # [128, 8]
        in_maps.append({
            "xT": xT_c, "wq": wq_c, "wk": wk_c, "wv": wv_c,
            "wot": wot_c, "bo2": bo2_c,
        })
    return in_maps


def kernel(x, Wq, Wk, Wv, Wo, bo):
    nc = _get_nc(with_rs=True)
    in_maps = make_in_maps(x, Wq, Wk, Wv, Wo, bo)
    # The axon-tunneled devices occasionally fail transiently
    # (NRT_EXEC_UNIT_UNRECOVERABLE / tunnel hangup); a retry recovers.
    last_err = None
    for _ in range(3):
        try:
            res = run_bass_kernel_spmd(nc, in_maps, list(range(N_CORES))).results
            break
        except Exception as e:  # noqa: BLE001
            last_err = e
            import time
            time.sleep(5)
    else:
        raise last_err

    out = np.empty((B, T, C), dtype=np.float32)
    for c in range(N_CORES):
        b, e = c // 2, c % 2
        yc = np.asarray(res[c]["y"], dtype=np.float32)  # [tt, c' slab, t]
        for tt in range(TTn):
            out[b, tt * 512:(tt + 1) * 512, e * 512:(e + 1) * 512] = yc[tt].T
    return out


# revision 27
# speedup vs baseline: 1.0202x; 1.0202x over previous
"""Multi-head causal attention (B=4, T=2048, C=1024, H=16, D=64) on 8 trn2 cores.

Sharding: tensor-parallel over heads within batch core-pairs.
  core c -> batch b = c//2, heads hoff..hoff+7 where hoff = (c%2)*8.

v2 design (all-bf16 dataflow):
  - Projections (Q^T/K^T per head-pair, V per key-chunk with a folded ones
    column) are software-pipelined INTO the attention loop as PE filler work:
    slab s+1's projection matmuls are interleaved between slab s's attention
    groups, because attention is ACT(exp)-bound while projections are pure PE.
  - Scores per head in S^T = [key, query] orientation, exp without
    max-subtraction (scores ~N(0, 0.25^2)), single merged causal
    affine_select per band group.
  - AV in O-orientation: stationary = exp(S^T) chunk [128k, 128q], moving =
    V [128k, 65] (col 64 = ones -> softmax sums land in ov col 64). 65-row
    bf16 matmuls halve AV PE time vs the O^T orientation.
  - Normalization: per-partition reciprocal + tensor_tensor multiply (queries
    are on partitions in O-layout), then O is transposed back to O^T via
    128-row matmuls against an identity for the output projection.
  - Output projection to partial y^T (+bo/2) in bf16; pairwise ReduceScatter
    (bf16 payload, 4 t-slabs) sums partner partials; core even keeps
    c' 0:512, odd keeps 512:1024.
Host reassembles the [B, T, C] f32 output by transposing/concatenating slabs.
"""

import numpy as np
import ml_dtypes

import concourse.bass as bass
import concourse.mybir as mybir
from concourse import bacc
from concourse.tile import TileContext
from concourse.bass_utils import run_bass_kernel_spmd

F32 = mybir.dt.float32
BF16 = mybir.dt.bfloat16

B, T, C = 4, 2048, 1024
H, D = 16, 64
HC = 8           # heads per core
NPAIR = HC // 2  # head pairs
CCn = C // 128   # 8 contraction chunks
TTn = T // 512   # 4 slabs of 512
JCn = T // 128   # 16 key chunks of 128
N_CORES = 8
RG = [[0, 1], [2, 3], [4, 5], [6, 7]]


def build_nc(with_rs: bool = True, debug: bool = False):
    nc = bacc.Bacc(None, target_bir_lowering=False)

    xT = nc.declare_dram_parameter("xT", [C, T], BF16, isOutput=False)
    wq = nc.declare_dram_parameter("wq", [C, 512], BF16, isOutput=False)
    wk = nc.declare_dram_parameter("wk", [C, 512], BF16, isOutput=False)
    wv = nc.declare_dram_parameter("wv", [C, 512], BF16, isOutput=False)
    wot = nc.declare_dram_parameter("wot", [512, C], BF16, isOutput=False)
    bo2 = nc.declare_dram_parameter("bo2", [128, 8], F32, isOutput=False)
    y = nc.declare_dram_parameter("y", [TTn, 512, 512], BF16, isOutput=True)
    if debug:
        dbg_kq = nc.declare_dram_parameter("dbg_kq", [2, 128, 512], BF16,
                                           isOutput=True)
        dbg_v = nc.declare_dram_parameter("dbg_v", [4, 128, 520], BF16,
                                          isOutput=True)
        dbg_pt = nc.declare_dram_parameter("dbg_pt", [2, 128, 1024], BF16,
                                           isOutput=True)
        dbg_ov = nc.declare_dram_parameter("dbg_ov", [128, 260], F32,
                                           isOutput=True)
        dbg_op = nc.declare_dram_parameter("dbg_op", [128, 512], BF16,
                                           isOutput=True)
        dbg_ot = nc.declare_dram_parameter("dbg_ot", [128, 512], BF16,
                                           isOutput=True)

    with TileContext(nc) as tc:
        with (
            tc.tile_pool(name="persist", bufs=1) as sb,
            tc.tile_pool(name="psum", bufs=1, space="PSUM") as psum,
            tc.tile_pool(name="dram", bufs=1, space="DRAM") as dram,
        ):
            # ---- persistent SBUF tiles (per-slab splits avoid false deps
            # between interleaved projection writes and attention reads) ----
            qt = {(p, s): sb.tile([128, 512], BF16, tag=f"qt{p}_{s}", name=f"qt{p}_{s}")
                  for p in range(NPAIR) for s in range(TTn)}
            kt = {(p, s): sb.tile([128, 512], BF16, tag=f"kt{p}_{s}", name=f"kt{p}_{s}")
                  for p in range(NPAIR) for s in range(TTn)}
            v = [sb.tile([128, 65 * HC], BF16, tag=f"v{j}", name=f"v{j}") for j in range(JCn)]
            ot = {(p, s): sb.tile([128, 512], BF16, tag=f"ot{p}_{s}", name=f"ot{p}_{s}")
                  for p in range(NPAIR) for s in range(TTn)}
            wqb = sb.tile([128, CCn * 512], BF16, tag="wqb", name="wqb")
            wkb = sb.tile([128, CCn * 512], BF16, tag="wkb", name="wkb")
            wvb = sb.tile([128, CCn * 512], BF16, tag="wvb", name="wvb")
            wob = sb.tile([128, 4 * C], BF16, tag="wob", name="wob")
            ones8 = sb.tile([128, HC], BF16, tag="ones8")
            ident = sb.tile([128, 128], BF16, tag="ident")
            bo_sb = sb.tile([128, 8], F32, tag="bo_sb")

            y_part = dram.tile([TTn, 1024, 512], BF16)
            rs_out = dram.tile([TTn, 512, 512], BF16)

            # ---- prologue DMAs: one batched strided transfer per tensor
            # (per-DMA queue overhead ~0.6us dominates small transfers) ----
            xtb_of = {}

            def issue_xts(s):
                i0 = s * 512
                t = sb.tile([128, CCn * 512], BF16, tag="xtb", bufs=2,
                            name=f"xtb{s}")
                nc.sync.dma_start(
                    out=t[:].rearrange("p (cc t) -> p cc t", cc=CCn),
                    in_=xT[:, i0:i0 + 512].rearrange(
                        "(cc p) t -> p cc t", cc=CCn))
                xtb_of[s] = t

            for hh in range(2):
                cs = slice(hh * 4 * 512, (hh + 1) * 4 * 512)
                rs_ = slice(hh * 4 * 128, (hh + 1) * 4 * 128)
                nc.sync.dma_start(
                    out=wkb[:, cs].rearrange("p (cc j) -> p cc j", cc=4),
                    in_=wk[rs_, :].rearrange("(cc p) j -> p cc j", cc=4))
                t0_ = sb.tile([128, CCn * 512], BF16, tag="xtb", bufs=2,
                              name="xtb0") if hh == 0 else xtb_of[0]
                xtb_of[0] = t0_
                nc.sync.dma_start(
                    out=t0_[:, cs].rearrange("p (cc t) -> p cc t", cc=4),
                    in_=xT[rs_, 0:512].rearrange("(cc p) t -> p cc t", cc=4))
            nc.sync.dma_start(
                out=wqb[:].rearrange("p (cc j) -> p cc j", cc=CCn),
                in_=wq[:].rearrange("(cc p) j -> p cc j", cc=CCn))
            nc.sync.dma_start(
                out=wvb[:].rearrange("p (cc j) -> p cc j", cc=CCn),
                in_=wv[:].rearrange("(cc p) j -> p cc j", cc=CCn))
            nc.sync.dma_start(
                out=wob[:].rearrange("p (cl j) -> p cl j", cl=4),
                in_=wot[:].rearrange("(cl p) j -> p cl j", cl=4))
            nc.sync.dma_start(out=bo_sb[:], in_=bo2[:])
            nc.vector.memset(ones8[:], 1.0)
            # identity for O -> O^T transposes: memset 1, keep only the diagonal
            nc.vector.memset(ident[:], 1.0)
            nc.gpsimd.affine_select(
                out=ident[:], in_=ident[:], compare_op=mybir.AluOpType.is_ge,
                fill=0.0, base=0, pattern=[[1, 128]], channel_multiplier=-1)
            nc.gpsimd.affine_select(
                out=ident[:], in_=ident[:], compare_op=mybir.AluOpType.is_ge,
                fill=0.0, base=0, pattern=[[-1, 128]], channel_multiplier=1)

            # ---- projection emission units (filler work for the PE) ----
            # Each unit emits ~2-4 matmuls (~0.4us of PE). A group's PSUM tile
            # is allocated by its first unit; the last unit emits the copy out.
            def make_proj_units(s, defer_qk_pairs=()):
                units = []
                deferred = []  # (unit, deadline_slot) for the NEXT slab
                xtb = xtb_of[s]

                def qk_group(wb, pair, dst_tile, defer_dl=None):
                    cell = {}
                    for cc0 in range(0, CCn, 2):
                        def u(cc0=cc0, wb=wb, pair=pair, cell=cell):
                            if cc0 == 0:
                                cell["ps"] = psum.tile([128, 512], F32,
                                                       tag="yps", bufs=2,
                                                       name="qkps")
                            ps = cell["ps"]
                            for cc in (cc0, cc0 + 1):
                                nc.tensor.matmul(
                                    ps[:],
                                    wb[:, cc * 512 + pair * 128:
                                       cc * 512 + pair * 128 + 128],
                                    xtb[:, cc * 512:(cc + 1) * 512],
                                    start=(cc == 0), stop=(cc == CCn - 1),
                                    skip_group_check=True)
                            if cc0 == CCn - 2:
                                nc.vector.tensor_copy(dst_tile[:], ps[:])
                        if defer_dl is None:
                            units.append(u)
                        else:
                            deferred.append((u, defer_dl))

                ng = 2 * (s + 1)  # groups/head in the consuming slab
                for p in range(NPAIR):
                    if p in defer_qk_pairs:
                        # needed by the consuming slab: Q at (h=2p, g0),
                        # K at (h=2p, first band group g=2s)
                        qk_group(wkb, p, kt[(p, s)], defer_dl=2 * ng * p + 2 * s - 1)
                        qk_group(wqb, p, qt[(p, s)], defer_dl=2 * ng * p - 1)
                    else:
                        qk_group(wkb, p, kt[(p, s)])
                        qk_group(wqb, p, qt[(p, s)])

                def v_group(jc):
                    jl = jc * 128 - s * 512
                    cell = {}
                    for i in range(4):
                        def u(i=i, jc=jc, jl=jl, cell=cell):
                            if i == 0:
                                cell["ps"] = psum.tile([128, 512], F32,
                                                       tag="yps", bufs=2, name="vps")
                            ps = cell["ps"]
                            for m in range(4):
                                gm = i * 4 + m
                                g, cc = gm // 8, gm % 8
                                nc.tensor.matmul(
                                    ps[:, g * 256:(g + 1) * 256],
                                    xtb[:, cc * 512 + jl:cc * 512 + jl + 128],
                                    wvb[:, cc * 512 + g * 256:
                                        cc * 512 + g * 256 + 256],
                                    start=(cc == 0), stop=(cc == CCn - 1),
                                    skip_group_check=True)
                            if i == 3:
                                vv = v[jc][:].rearrange(
                                    "p (h e) -> p h e", h=HC, e=65)
                                nc.vector.tensor_copy(vv[:, :, 0:64], ps[:])
                                nc.vector.tensor_copy(vv[:, :, 64:65],
                                                      ones8[:])
                        units.append(u)

                for jc in range(4 * s, 4 * s + 4):
                    v_group(jc)
                deferred.sort(key=lambda t: t[1])
                return units, deferred

            # interleave order inside PROJ(0) so tt0/h0 attention can start
            # as early as possible: K0,Q0,V0,V1 then the rest
            units0, _ = make_proj_units(0)
            # units0 layout: [K0(4), Q0(4), K1(4), Q1(4), ... V groups(4x4)]
            order0 = (units0[0:8] + units0[32:40] + units0[8:16]
                      + units0[40:48] + units0[16:32])
            for u in order0:
                u()
            deferred_next = []

            # ---- attention + interleaved projections ----
            pt_pool = sb
            held = None          # pending AV emission for the previous group
            pending_norm = None  # (tt, h, ov) awaiting recip+TT
            opair_cell = {}      # pair -> o_pair staging tile

            def emit_avs(hd):
                tt_, h_, pt_, a_of = hd
                ovt = ov_of[(tt_, h_)]
                for qq in range(4):
                    for idx, (jc, a) in enumerate(a_of):
                        if a <= qq * 128:
                            c0 = idx * 512 + qq * 128
                            # start only once per head: start=True arms a
                            # pending-zero over the whole 2KB PSUM zero
                            # region, so later qq slices first-touch-replace
                            # rather than re-arm (which would wipe earlier
                            # slices' partials on their next accumulate).
                            nc.tensor.matmul(
                                ovt[:, qq * 65:qq * 65 + 65],
                                pt_[:, c0:c0 + 128],
                                v[jc][:, h_ * 65:(h_ + 1) * 65],
                                start=(jc == 0 and qq == 0),
                                stop=(jc == 4 * tt_ + qq),
                                skip_group_check=True)

            ov_of = {}

            def emit_norm(tt, h, ov):
                p, e = h // 2, h % 2
                if e == 0:
                    opair_cell[p] = sb.tile([128, 512], BF16, tag="opair",
                                            bufs=2, name=f"op{tt}{p}")
                opair = opair_cell[p]
                ovr = ov[:].rearrange("p (q o e) -> p q o e", q=4, o=1, e=65)
                rl = sb.tile([128, 4], F32, tag="rl", bufs=2, name="rl")
                nc.vector.reciprocal(
                    rl[:].rearrange("p (q o) -> p q o", q=4, o=1),
                    ov[:].rearrange("p (q e) -> p q e", q=4, e=65)[:, :, 64:65])
                opr = opair[:].rearrange("p (q hh e) -> p q hh e",
                                         q=4, hh=2, e=64)
                nc.vector.tensor_mul(
                    opr[:, :, e:e + 1, :], ovr[:, :, :, 0:64],
                    rl[:].rearrange("p (q o u) -> p q o u", q=4, o=1, u=1)
                    .broadcast_to((128, 4, 1, 64)))
                if debug and (tt, h) == (0, 0):
                    ov_stage = sb.tile([128, 260], F32, tag="dbgov",
                                       name="dbgov")
                    nc.vector.tensor_copy(ov_stage[:], ov[:])
                    nc.sync.dma_start(out=dbg_ov[:], in_=ov_stage[:])
                if e == 1:
                    # pair complete: transpose O -> O^T into ot[(p, tt)]
                    trp = psum.tile([128, 512], F32, tag="yps", bufs=2,
                                    name=f"tr{tt}{p}")
                    for qq in range(4):
                        nc.tensor.matmul(
                            trp[:, qq * 128:(qq + 1) * 128],
                            opair[:, qq * 128:(qq + 1) * 128],
                            ident[:], start=True, stop=True,
                            skip_group_check=True)
                    nc.vector.tensor_copy(ot[(p, tt)][:], trp[:])
                    if debug and (tt, h) == (0, 1):
                        nc.sync.dma_start(out=dbg_op[:], in_=opair[:])
                        nc.sync.dma_start(out=dbg_ot[:], in_=ot[(p, tt)][:])
                    del opair_cell[p]

            pg_tiles = {}

            def emit_partial_outproj(cp):
                # cl 0..2 partial sum for the final slab, staged to SBUF so
                # the epilogue only needs the cl=3 matmul + one combine
                yps = psum.tile([128, 512], F32, tag="yps", bufs=2,
                                name=f"pg{cp}")
                for cl in range(3):
                    nc.tensor.matmul(
                        yps[:], wob[:, cl * C + cp * 128:cl * C + cp * 128 + 128],
                        ot[(cl, TTn - 1)][:], start=(cl == 0), stop=(cl == 2),
                        skip_group_check=True)
                pg = sb.tile([128, 512], F32, tag=f"pg{cp}", name=f"pgs{cp}")
                nc.vector.tensor_copy(pg[:], yps[:])
                pg_tiles[cp] = pg

            def emit_outproj_group(tt, cp):
                yps = psum.tile([128, 512], F32, tag="yps", bufs=2,
                                name=f"yps{tt}{cp}")
                pg = pg_tiles.get(cp) if tt == TTn - 1 else None
                cl0 = 3 if pg is not None else 0
                for cl in range(cl0, 4):
                    nc.tensor.matmul(
                        yps[:], wob[:, cl * C + cp * 128:cl * C + cp * 128 + 128],
                        ot[(cl, tt)][:], start=(cl == cl0), stop=(cl == 3),
                        skip_group_check=True)
                ysb = sb.tile([128, 512], BF16, tag="ysb", bufs=4, name="ysb")
                if pg is not None:
                    nc.vector.scalar_tensor_tensor(
                        ysb[:], yps[:], bo_sb[:, cp:cp + 1], pg[:],
                        mybir.AluOpType.add, mybir.AluOpType.add)
                else:
                    nc.vector.tensor_scalar_add(ysb[:], yps[:],
                                                bo_sb[:, cp:cp + 1])
                nc.sync.dma_start(
                    out=y_part[tt, cp * 128:(cp + 1) * 128, :], in_=ysb[:])

            def emit_rs(tt):
                if with_rs:
                    nc.gpsimd.collective_compute(
                        "ReduceScatter", mybir.AluOpType.add,
                        replica_groups=RG,
                        ins=[y_part[tt]], outs=[rs_out[tt]])
                    nc.sync.dma_start(out=y[tt], in_=rs_out[tt])
                else:
                    nc.sync.dma_start(out=y[tt], in_=y_part[tt, 0:512, :])

            for tt in range(TTn):
                i0 = tt * 512
                n_g = 2 * (tt + 1)
                deferred = deferred_next  # deferred here from the prior make
                if tt < TTn - 1:
                    issue_xts(tt + 1)
                    proj_units, deferred_next = make_proj_units(
                        tt + 1, defer_qk_pairs=(1, 2, 3) if tt == 2 else ())
                else:
                    proj_units = []
                    deferred_next = []
                slot_idx = [0]
                emitted = [0]
                total_slots = 8 * n_g
                total_def = len(deferred)

                def emit_filler():
                    si = slot_idx[0]
                    slot_idx[0] += 1
                    rem_slots = total_slots - si
                    if proj_units and rem_slots > 0:
                        n = -(-len(proj_units) // rem_slots)  # ceil
                        for _ in range(min(n, 8)):
                            if proj_units:
                                proj_units.pop(0)()
                    # deferred units: emit when due (deadline) or to keep
                    # proportional pace across the whole slab
                    target = (si + 1) * total_def // max(total_slots, 1)
                    while deferred and (deferred[0][1] <= si + 1
                                        or emitted[0] < target):
                        deferred.pop(0)[0]()
                        emitted[0] += 1

                for h in range(HC):
                    p, e = h // 2, h % 2
                    ov_of[(tt, h)] = psum.tile([128, 260], F32, tag="ovps",
                                               bufs=2, name=f"ov{tt}{h}")
                    for g in range(n_g):
                        jc0, jc1 = 2 * g, 2 * g + 1
                        a0 = max(0, (jc0 - 4 * tt)) * 128
                        a1 = max(0, (jc1 - 4 * tt)) * 128
                        st = psum.tile([128, 1024], F32, tag="stps", bufs=2,
                                       name=f"st{tt}{h}{g}")
                        for k, (jc, a) in enumerate(((jc0, a0), (jc1, a1))):
                            nc.tensor.matmul(
                                st[:, k * 512 + a:(k + 1) * 512],
                                kt[(p, jc // 4)][e * 64:(e + 1) * 64,
                                                 (jc % 4) * 128:
                                                 (jc % 4) * 128 + 128],
                                qt[(p, tt)][e * 64:(e + 1) * 64, a:512],
                                start=True, stop=True, skip_group_check=True)
                        if held is not None:
                            emit_avs(held)
                            held = None
                        if g == 0 and tt >= 1 and h in (1, 2, 3, 4):
                            for g2 in range(2):
                                emit_outproj_group(tt - 1, 2 * (h - 1) + g2)
                            if h == 4:
                                emit_rs(tt - 1)
                        if g == 1 and pending_norm is not None:
                            emit_norm(*pending_norm)
                            pending_norm = None
                        if (tt == TTn - 1 and h == 7 and len(pg_tiles) < 8):
                            emit_partial_outproj(len(pg_tiles))
                        emit_filler()
                        pt = pt_pool.tile([128, 1024], BF16, tag="pt", bufs=7,
                                          name=f"pt{tt}{h}{g}")
                        if debug and tt == 0 and h == 0:
                            nc.vector.memset(pt[:], 0.0)
                        if jc0 >= 4 * tt:
                            # band group: exact per-chunk exp + causal zeroing
                            # (regions outside [k*512+a, (k+1)*512) are never
                            # read downstream, so they stay unwritten)
                            for k, a in ((0, a0), (1, a1)):
                                lo, hi = k * 512 + a, (k + 1) * 512
                                nc.scalar.activation(
                                    pt[:, lo:hi], st[:, lo:hi],
                                    mybir.ActivationFunctionType.Exp)
                                nc.gpsimd.affine_select(
                                    out=pt[:, lo:hi], in_=pt[:, lo:hi],
                                    compare_op=mybir.AluOpType.is_ge,
                                    fill=0.0, base=0,
                                    pattern=[[1, 512 - a]],
                                    channel_multiplier=-1)
                        else:
                            nc.scalar.activation(
                                pt[:, 0:1024], st[:, 0:1024],
                                mybir.ActivationFunctionType.Exp)
                        if debug and tt == 0 and h == 0:
                            nc.sync.dma_start(out=dbg_pt[g], in_=pt[:])
                            if g == 0:
                                nc.sync.dma_start(out=dbg_kq[0],
                                                  in_=kt[(0, 0)][:])
                                nc.sync.dma_start(out=dbg_kq[1],
                                                  in_=qt[(0, 0)][:])
                                for _j in range(4):
                                    nc.sync.dma_start(out=dbg_v[_j],
                                                      in_=v[_j][:])
                        held = (tt, h, pt, ((jc0, a0), (jc1, a1)))
                    if pending_norm is not None:  # tt0: only 2 groups per head
                        emit_norm(*pending_norm)
                        pending_norm = None
                    pending_norm = (tt, h, ov_of[(tt, h)])
                while proj_units:
                    proj_units.pop(0)()
                while deferred:
                    deferred.pop(0)[0]()
                if tt == TTn - 1:
                    if held is not None:
                        emit_avs(held)
                        held = None
                    if pending_norm is not None:
                        emit_norm(*pending_norm)
                        pending_norm = None
                    for cp in range(8):
                        emit_outproj_group(tt, cp)
                    emit_rs(tt)

    nc.compile()
    return nc


_NC_CACHE = {}


def _get_nc(with_rs: bool = True):
    key = bool(with_rs)
    if key not in _NC_CACHE:
        _NC_CACHE[key] = build_nc(with_rs)
    return _NC_CACHE[key]


def make_in_maps(x, Wq, Wk, Wv, Wo, bo):
    bf16 = ml_dtypes.bfloat16
    x = np.asarray(x, dtype=np.float32)
    Wq = np.asarray(Wq, dtype=np.float32)
    Wk = np.asarray(Wk, dtype=np.float32)
    Wv = np.asarray(Wv, dtype=np.float32)
    Wo = np.asarray(Wo, dtype=np.float32)
    bo = np.asarray(bo, dtype=np.float32)

    scale = np.float32(C) ** np.float32(-0.5)
    in_maps = []
    for c in range(N_CORES):
        b, hoff = c // 2, (c % 2) * HC
        heads = slice(hoff, hoff + HC)
        xT_c = np.ascontiguousarray(x[b].T).astype(bf16)             # [C, T]
        wq_c = np.ascontiguousarray(
            np.concatenate(list(Wq[heads] * scale), axis=1)).astype(bf16)
        wk_c = np.ascontiguousarray(
            np.concatenate(list(Wk[heads]), axis=1)).astype(bf16)
        wv_c = np.ascontiguousarray(
            np.concatenate(list(Wv[heads]), axis=1)).astype(bf16)
        wot_c = np.ascontiguousarray(
            Wo[:, hoff * D:(hoff + HC) * D].T).astype(bf16)          # [512, C]
        bo2_c = np.ascontiguousarray((bo / 2.0).reshape(8, 128).T)   # [128, 8]
        in_maps.append({
            "xT": xT_c, "wq": wq_c, "wk": wk_c, "wv": wv_c,
            "wot": wot_c, "bo2": bo2_c,
        })
    return in_maps


def kernel(x, Wq, Wk, Wv, Wo, bo):
    nc = _get_nc(with_rs=True)
    in_maps = make_in_maps(x, Wq, Wk, Wv, Wo, bo)
    # The axon-tunneled devices occasionally fail transiently
    # (NRT_EXEC_UNIT_UNRECOVERABLE / tunnel hangup); a retry recovers.
    last_err = None
    for _ in range(3):
        try:
            res = run_bass_kernel_spmd(nc, in_maps, list(range(N_CORES))).results
            break
        except Exception as e:  # noqa: BLE001
            last_err = e
            import time
            time.sleep(5)
    else:
        raise last_err

    out = np.empty((B, T, C), dtype=np.float32)
    for c in range(N_CORES):
        b, e = c // 2, c % 2
        yc = np.asarray(res[c]["y"], dtype=np.float32)  # [tt, c' slab, t]
        for tt in range(TTn):
            out[b, tt * 512:(tt + 1) * 512, e * 512:(e + 1) * 512] = yc[tt].T
    return out


# revision 37
# speedup vs baseline: 1.0571x; 1.0362x over previous
"""Multi-head causal attention (B=4, T=2048, C=1024, H=16, D=64) on 8 trn2 cores.

Sharding: tensor-parallel over heads within batch core-pairs.
  core c -> batch b = c//2, heads hoff..hoff+7 where hoff = (c%2)*8.

v2 design (all-bf16 dataflow; 377.8us baseline -> 284.2us):
  - Projections (Q^T/K^T per head-pair, V per key-chunk with a folded ones
    column) are software-pipelined INTO the attention loop as PE filler work:
    slab s+1's projection matmuls are paced between slab s's attention
    groups, because attention is ACT(exp)-bound while projections are pure
    PE. Inputs arrive as a handful of large batched strided DMAs (per-DMA
    queue overhead ~0.6us dominates small transfers).
  - Attention emission is organized in 4 windows whose head lists pull the
    first two heads of the next slab forward (their exps fill the ACT-idle
    tail of the previous window); within a head, the diagonal band groups
    run FIRST and off-band groups last, so the next head's scores never
    stall on a bunched-up chain of band exps.
  - Scores per head in S^T = [key, query] orientation, exp without
    max-subtraction (scores ~N(0, 0.25^2)), exact per-chunk exp ranges and
    per-chunk causal affine_selects (never touching unwritten PSUM).
  - AV in O-orientation: stationary = exp(S^T) chunk [128k, 128q], moving =
    V [128k, 65] (col 64 = ones -> softmax sums land in ov col 64). 65-row
    bf16 matmuls halve AV PE time vs the O^T orientation. PSUM zero-region
    note: start=True arms pending-zero for the whole 2KB region, so it is
    issued exactly once per head (first AV); later qq slices first-touch-
    replace and then accumulate.
  - Normalization: per-partition reciprocal + tensor_tensor multiply (queries
    are on partitions in O-layout), then O is transposed back to O^T via
    128-row matmuls against an identity for the output projection.
  - Output projection to partial y^T (+bo/2) in bf16; pairwise ReduceScatter
    (bf16 payload halves the 15us+bytes/40GBps collective cost; 4 t-slabs)
    sums partner partials; core even keeps c' 0:512, odd keeps 512:1024.
    The final slab's outproj pre-accumulates head-pairs 0..2 during the last
    head's slots so only the cl=3 matmul + one combine sit before the last
    (fully exposed) ReduceScatter.
Host reassembles the [B, T, C] f32 output by transposing/concatenating slabs.
"""

import numpy as np
import ml_dtypes

import concourse.bass as bass
import concourse.mybir as mybir
from concourse import bacc
from concourse.tile import TileContext
from concourse.bass_utils import run_bass_kernel_spmd

F32 = mybir.dt.float32
BF16 = mybir.dt.bfloat16

B, T, C = 4, 2048, 1024
H, D = 16, 64
HC = 8           # heads per core
NPAIR = HC // 2  # head pairs
CCn = C // 128   # 8 contraction chunks
TTn = T // 512   # 4 slabs of 512
JCn = T // 128   # 16 key chunks of 128
N_CORES = 8
RG = [[0, 1], [2, 3], [4, 5], [6, 7]]


def build_nc(with_rs: bool = True, debug: bool = False):
    nc = bacc.Bacc(None, target_bir_lowering=False)

    xT = nc.declare_dram_parameter("xT", [C, T], BF16, isOutput=False)
    wq = nc.declare_dram_parameter("wq", [C, 512], BF16, isOutput=False)
    wk = nc.declare_dram_parameter("wk", [C, 512], BF16, isOutput=False)
    wv = nc.declare_dram_parameter("wv", [C, 512], BF16, isOutput=False)
    wot = nc.declare_dram_parameter("wot", [512, C], BF16, isOutput=False)
    bo2 = nc.declare_dram_parameter("bo2", [128, 8], F32, isOutput=False)
    y = nc.declare_dram_parameter("y", [TTn, 512, 512], BF16, isOutput=True)
    if debug:
        dbg_kq = nc.declare_dram_parameter("dbg_kq", [2, 128, 512], BF16,
                                           isOutput=True)
        dbg_v = nc.declare_dram_parameter("dbg_v", [4, 128, 520], BF16,
                                          isOutput=True)
        dbg_pt = nc.declare_dram_parameter("dbg_pt", [2, 128, 1024], BF16,
                                           isOutput=True)
        dbg_ov = nc.declare_dram_parameter("dbg_ov", [128, 260], F32,
                                           isOutput=True)
        dbg_op = nc.declare_dram_parameter("dbg_op", [128, 512], BF16,
                                           isOutput=True)
        dbg_ot = nc.declare_dram_parameter("dbg_ot", [128, 512], BF16,
                                           isOutput=True)

    with TileContext(nc) as tc:
        with (
            tc.tile_pool(name="persist", bufs=1) as sb,
            tc.tile_pool(name="psum", bufs=1, space="PSUM") as psum,
            tc.tile_pool(name="dram", bufs=1, space="DRAM") as dram,
        ):
            # ---- persistent SBUF tiles (per-slab splits avoid false deps
            # between interleaved projection writes and attention reads) ----
            qt = {(p, s): sb.tile([128, 512], BF16, tag=f"qt{p}_{s}", name=f"qt{p}_{s}")
                  for p in range(NPAIR) for s in range(TTn)}
            kt = {(p, s): sb.tile([128, 512], BF16, tag=f"kt{p}_{s}", name=f"kt{p}_{s}")
                  for p in range(NPAIR) for s in range(TTn)}
            v = [sb.tile([128, 65 * HC], BF16, tag=f"v{j}", name=f"v{j}") for j in range(JCn)]
            ot = {(p, s): sb.tile([128, 512], BF16, tag=f"ot{p}_{s}", name=f"ot{p}_{s}")
                  for p in range(NPAIR) for s in range(TTn)}
            wqb = sb.tile([128, CCn * 512], BF16, tag="wqb", name="wqb")
            wkb = sb.tile([128, CCn * 512], BF16, tag="wkb", name="wkb")
            wvb = sb.tile([128, CCn * 512], BF16, tag="wvb", name="wvb")
            wob = sb.tile([128, 4 * C], BF16, tag="wob", name="wob")
            ones8 = sb.tile([128, HC], BF16, tag="ones8")
            ident = sb.tile([128, 128], BF16, tag="ident")
            bo_sb = sb.tile([128, 8], F32, tag="bo_sb")

            y_part = dram.tile([TTn, 1024, 512], BF16)
            rs_out = dram.tile([TTn, 512, 512], BF16)

            # ---- prologue DMAs: one batched strided transfer per tensor
            # (per-DMA queue overhead ~0.6us dominates small transfers) ----
            xtb_of = {}

            def issue_xts(s):
                i0 = s * 512
                t = sb.tile([128, CCn * 512], BF16, tag="xtb", bufs=2,
                            name=f"xtb{s}")
                nc.sync.dma_start(
                    out=t[:].rearrange("p (cc t) -> p cc t", cc=CCn),
                    in_=xT[:, i0:i0 + 512].rearrange(
                        "(cc p) t -> p cc t", cc=CCn))
                xtb_of[s] = t

            t0_ = sb.tile([128, CCn * 512], BF16, tag="xtb", bufs=2,
                          name="xtb0")
            xtb_of[0] = t0_
            for hh in range(4):
                cs = slice(hh * 2 * 512, (hh + 1) * 2 * 512)
                rs_ = slice(hh * 2 * 128, (hh + 1) * 2 * 128)
                nc.sync.dma_start(
                    out=wkb[:, cs].rearrange("p (cc j) -> p cc j", cc=2),
                    in_=wk[rs_, :].rearrange("(cc p) j -> p cc j", cc=2))
                nc.sync.dma_start(
                    out=t0_[:, cs].rearrange("p (cc t) -> p cc t", cc=2),
                    in_=xT[rs_, 0:512].rearrange("(cc p) t -> p cc t", cc=2))
                nc.sync.dma_start(
                    out=wqb[:, cs].rearrange("p (cc j) -> p cc j", cc=2),
                    in_=wq[rs_, :].rearrange("(cc p) j -> p cc j", cc=2))
            nc.sync.dma_start(
                out=wvb[:].rearrange("p (cc j) -> p cc j", cc=CCn),
                in_=wv[:].rearrange("(cc p) j -> p cc j", cc=CCn))
            nc.sync.dma_start(
                out=wob[:].rearrange("p (cl j) -> p cl j", cl=4),
                in_=wot[:].rearrange("(cl p) j -> p cl j", cl=4))
            nc.sync.dma_start(out=bo_sb[:], in_=bo2[:])
            nc.vector.memset(ones8[:], 1.0)
            # identity for O -> O^T transposes: memset 1, keep only the diagonal
            nc.vector.memset(ident[:], 1.0)
            nc.gpsimd.affine_select(
                out=ident[:], in_=ident[:], compare_op=mybir.AluOpType.is_ge,
                fill=0.0, base=0, pattern=[[1, 128]], channel_multiplier=-1)
            nc.gpsimd.affine_select(
                out=ident[:], in_=ident[:], compare_op=mybir.AluOpType.is_ge,
                fill=0.0, base=0, pattern=[[-1, 128]], channel_multiplier=1)

            # ---- projection emission units (filler work for the PE) ----
            # Each unit emits ~2-4 matmuls (~0.4us of PE). A group's PSUM tile
            # is allocated by its first unit; the last unit emits the copy out.
            def make_proj_units(s, defer_qk_pairs=()):
                units = []
                unit_chunks = []   # list of per-group unit lists, woven below
                deferred = []  # (unit, deadline_slot) for the NEXT slab
                xtb = xtb_of[s]

                def qk_group(wb, pair, dst_tile, defer_dl=None):
                    cell = {}
                    for cc0 in range(0, CCn, 2):
                        def u(cc0=cc0, wb=wb, pair=pair, cell=cell):
                            if cc0 == 0:
                                cell["ps"] = psum.tile([128, 512], F32,
                                                       tag="yps", bufs=2,
                                                       name="qkps")
                            ps = cell["ps"]
                            for cc in (cc0, cc0 + 1):
                                nc.tensor.matmul(
                                    ps[:],
                                    wb[:, cc * 512 + pair * 128:
                                       cc * 512 + pair * 128 + 128],
                                    xtb[:, cc * 512:(cc + 1) * 512],
                                    start=(cc == 0), stop=(cc == CCn - 1),
                                    skip_group_check=True)
                            if cc0 == CCn - 2:
                                nc.vector.tensor_copy(dst_tile[:], ps[:])
                        if defer_dl is None:
                            cur_chunk.append(u)
                        else:
                            deferred.append((u, defer_dl))

                ng = 2 * (s + 1)  # groups/head in the consuming slab
                for p in range(NPAIR):
                    cur_chunk = []
                    if p in defer_qk_pairs:
                        qk_group(wkb, p, kt[(p, s)], defer_dl=10 ** 9)
                        qk_group(wqb, p, qt[(p, s)], defer_dl=10 ** 9)
                    else:
                        qk_group(wkb, p, kt[(p, s)])
                        qk_group(wqb, p, qt[(p, s)])
                    unit_chunks.append(cur_chunk)

                def v_group(jc):
                    jl = jc * 128 - s * 512
                    cell = {}
                    for i in range(4):
                        def u(i=i, jc=jc, jl=jl, cell=cell):
                            if i == 0:
                                cell["ps"] = psum.tile([128, 512], F32,
                                                       tag="yps", bufs=2, name="vps")
                            ps = cell["ps"]
                            for m in range(4):
                                gm = i * 4 + m
                                g, cc = gm // 8, gm % 8
                                nc.tensor.matmul(
                                    ps[:, g * 256:(g + 1) * 256],
                                    xtb[:, cc * 512 + jl:cc * 512 + jl + 128],
                                    wvb[:, cc * 512 + g * 256:
                                        cc * 512 + g * 256 + 256],
                                    start=(cc == 0), stop=(cc == CCn - 1),
                                    skip_group_check=True)
                            if i == 3:
                                vv = v[jc][:].rearrange(
                                    "p (h e) -> p h e", h=HC, e=65)
                                nc.vector.tensor_copy(vv[:, :, 0:64], ps[:])
                                nc.vector.tensor_copy(vv[:, :, 64:65],
                                                      ones8[:])
                        cur_chunk.append(u)

                for jc in range(4 * s, 4 * s + 4):
                    cur_chunk = []
                    v_group(jc)
                    unit_chunks.append(cur_chunk)
                # weave: K0Q0, V0, V1, K1Q1, V2, V3, K2Q2, K3Q3 — V chunks
                # early enough that pulled-forward heads of slab s (processed
                # late in window s-1) see their v[] tiles written in time
                qks, vs = unit_chunks[:NPAIR], unit_chunks[NPAIR:]
                for i, chunk in enumerate([qks[0], vs[0], vs[1], qks[1],
                                           vs[2], vs[3], qks[2], qks[3]]):
                    units.extend(chunk)
                deferred.sort(key=lambda t: t[1])
                return units, deferred

            # interleave order inside PROJ(0) so tt0/h0 attention can start
            # as early as possible: K0,Q0,V0,V1 then the rest
            units0, _ = make_proj_units(0)
            # units0 layout: [K0(4), Q0(4), K1(4), Q1(4), ... V groups(4x4)]
            order0 = (units0[0:8] + units0[32:40] + units0[8:16]
                      + units0[40:48] + units0[16:32])
            for u in order0:
                u()
            deferred_next = []

            # ---- attention + interleaved projections ----
            pt_pool = sb
            held = None          # pending AV emission for the previous group
            pending_norm = None  # (tt, h, ov) awaiting recip+TT
            opair_cell = {}      # pair -> o_pair staging tile

            def emit_avs(hd):
                tt_, h_, pt_, a_of = hd
                ovt = ov_of[(tt_, h_)]
                for qq in range(4):
                    for idx, (jc, a) in enumerate(a_of):
                        if a <= qq * 128:
                            c0 = idx * 512 + qq * 128
                            # start only once per head: start=True arms a
                            # pending-zero over the whole 2KB PSUM zero
                            # region, so later qq slices first-touch-replace
                            # rather than re-arm (which would wipe earlier
                            # slices' partials on their next accumulate).
                            nc.tensor.matmul(
                                ovt[:, qq * 65:qq * 65 + 65],
                                pt_[:, c0:c0 + 128],
                                v[jc][:, h_ * 65:(h_ + 1) * 65],
                                start=(jc == 4 * tt_ and qq == 0),
                                stop=(jc == 4 * tt_ - 1) if tt_ >= 1
                                else (jc == qq),
                                skip_group_check=True)

            ov_of = {}

            def emit_norm(tt, h, ov):
                p, e = h // 2, h % 2
                if e == 0:
                    opair_cell[p] = sb.tile([128, 512], BF16, tag="opair",
                                            bufs=2, name=f"op{tt}{p}")
                opair = opair_cell[p]
                ovr = ov[:].rearrange("p (q o e) -> p q o e", q=4, o=1, e=65)
                rl = sb.tile([128, 4], F32, tag="rl", bufs=2, name="rl")
                nc.vector.reciprocal(
                    rl[:].rearrange("p (q o) -> p q o", q=4, o=1),
                    ov[:].rearrange("p (q e) -> p q e", q=4, e=65)[:, :, 64:65])
                opr = opair[:].rearrange("p (q hh e) -> p q hh e",
                                         q=4, hh=2, e=64)
                nc.vector.tensor_mul(
                    opr[:, :, e:e + 1, :], ovr[:, :, :, 0:64],
                    rl[:].rearrange("p (q o u) -> p q o u", q=4, o=1, u=1)
                    .broadcast_to((128, 4, 1, 64)))
                if debug and (tt, h) == (0, 0):
                    ov_stage = sb.tile([128, 260], F32, tag="dbgov",
                                       name="dbgov")
                    nc.vector.tensor_copy(ov_stage[:], ov[:])
                    nc.sync.dma_start(out=dbg_ov[:], in_=ov_stage[:])
                if e == 1:
                    # pair complete: transpose O -> O^T into ot[(p, tt)]
                    trp = psum.tile([128, 512], F32, tag="yps", bufs=2,
                                    name=f"tr{tt}{p}")
                    for qq in range(4):
                        nc.tensor.matmul(
                            trp[:, qq * 128:(qq + 1) * 128],
                            opair[:, qq * 128:(qq + 1) * 128],
                            ident[:], start=True, stop=True,
                            skip_group_check=True)
                    nc.vector.tensor_copy(ot[(p, tt)][:], trp[:])
                    if debug and (tt, h) == (0, 1):
                        nc.sync.dma_start(out=dbg_op[:], in_=opair[:])
                        nc.sync.dma_start(out=dbg_ot[:], in_=ot[(p, tt)][:])
                    del opair_cell[p]

            pg_tiles = {}

            def emit_partial_outproj(cp):
                # cl 0..2 partial sum for the final slab, staged to SBUF so
                # the epilogue only needs the cl=3 matmul + one combine
                yps = psum.tile([128, 512], F32, tag="yps", bufs=2,
                                name=f"pg{cp}")
                for cl in range(3):
                    nc.tensor.matmul(
                        yps[:], wob[:, cl * C + cp * 128:cl * C + cp * 128 + 128],
                        ot[(cl, TTn - 1)][:], start=(cl == 0), stop=(cl == 2),
                        skip_group_check=True)
                pg = sb.tile([128, 512], F32, tag=f"pg{cp}", name=f"pgs{cp}")
                nc.vector.tensor_copy(pg[:], yps[:])
                pg_tiles[cp] = pg

            def emit_outproj_group(tt, cp):
                yps = psum.tile([128, 512], F32, tag="yps", bufs=2,
                                name=f"yps{tt}{cp}")
                pg = pg_tiles.get(cp) if tt == TTn - 1 else None
                cl0 = 3 if pg is not None else 0
                for cl in range(cl0, 4):
                    nc.tensor.matmul(
                        yps[:], wob[:, cl * C + cp * 128:cl * C + cp * 128 + 128],
                        ot[(cl, tt)][:], start=(cl == cl0), stop=(cl == 3),
                        skip_group_check=True)
                ysb = sb.tile([128, 512], BF16, tag="ysb", bufs=8, name="ysb")
                if pg is not None:
                    nc.vector.scalar_tensor_tensor(
                        ysb[:], yps[:], bo_sb[:, cp:cp + 1], pg[:],
                        mybir.AluOpType.add, mybir.AluOpType.add)
                else:
                    nc.vector.tensor_scalar_add(ysb[:], yps[:],
                                                bo_sb[:, cp:cp + 1])
                nc.sync.dma_start(
                    out=y_part[tt, cp * 128:(cp + 1) * 128, :], in_=ysb[:])

            def emit_rs(tt):
                if with_rs:
                    nc.gpsimd.collective_compute(
                        "ReduceScatter", mybir.AluOpType.add,
                        replica_groups=RG,
                        ins=[y_part[tt]], outs=[rs_out[tt]])
                    nc.sync.dma_start(out=y[tt], in_=rs_out[tt])
                else:
                    nc.sync.dma_start(out=y[tt], in_=y_part[tt, 0:512, :])

            windows = [
                [(0, h) for h in range(HC)],
                [(1, h) for h in range(HC)] + [(2, 0), (2, 1)],
                [(2, h) for h in range(2, HC)] + [(3, 0), (3, 1)],
                [(3, h) for h in range(2, HC)],
            ]
            for w in range(TTn):
                head_list = windows[w]
                deferred = deferred_next  # deferred here from the prior make
                if w < TTn - 1:
                    issue_xts(w + 1)
                    proj_units, deferred_next = make_proj_units(
                        w + 1, defer_qk_pairs=())
                else:
                    proj_units = []
                    deferred_next = []
                slot_idx = [0]
                emitted = [0]
                total_slots = sum(2 * (t_ + 1) for t_, _ in head_list)
                total_def = len(deferred)

                def emit_filler():
                    si = slot_idx[0]
                    slot_idx[0] += 1
                    rem_slots = total_slots - si
                    if proj_units and rem_slots > 0:
                        n = -(-len(proj_units) // rem_slots)  # ceil
                        for _ in range(min(n, 8)):
                            if proj_units:
                                proj_units.pop(0)()
                    # deferred units: emit when due (deadline) or to keep
                    # proportional pace across the whole slab
                    target = (si + 1) * total_def // max(total_slots, 1)
                    while deferred and (deferred[0][1] <= si + 1
                                        or emitted[0] < target):
                        deferred.pop(0)[0]()
                        emitted[0] += 1

                for tt, h in head_list:
                    i0 = tt * 512
                    n_g = 2 * (tt + 1)
                    g_seq = [2 * tt, 2 * tt + 1] + list(range(2 * tt))
                    p, e = h // 2, h % 2
                    ov_of[(tt, h)] = psum.tile([128, 260], F32, tag="ovps",
                                               bufs=2, name=f"ov{tt}{h}")
                    for gi, g in enumerate(g_seq):
                        jc0, jc1 = 2 * g, 2 * g + 1
                        a0 = max(0, (jc0 - 4 * tt)) * 128
                        a1 = max(0, (jc1 - 4 * tt)) * 128
                        st = psum.tile([128, 1024], F32, tag="stps", bufs=2,
                                       name=f"st{tt}{h}{g}")
                        for k, (jc, a) in enumerate(((jc0, a0), (jc1, a1))):
                            nc.tensor.matmul(
                                st[:, k * 512 + a:(k + 1) * 512],
                                kt[(p, jc // 4)][e * 64:(e + 1) * 64,
                                                 (jc % 4) * 128:
                                                 (jc % 4) * 128 + 128],
                                qt[(p, tt)][e * 64:(e + 1) * 64, a:512],
                                start=True, stop=True, skip_group_check=True)
                        if held is not None:
                            emit_avs(held)
                            held = None
                        if gi == 0 and tt >= 1 and h in (1, 2, 3, 4):
                            for g2 in range(2):
                                emit_outproj_group(tt - 1, 2 * (h - 1) + g2)
                            if h == 4:
                                emit_rs(tt - 1)
                        if gi == 1 and pending_norm is not None:
                            emit_norm(*pending_norm)
                            pending_norm = None
                        if (tt == TTn - 1 and h == 7 and len(pg_tiles) < 8):
                            emit_partial_outproj(len(pg_tiles))
                        emit_filler()
                        pt = pt_pool.tile([128, 1024], BF16, tag="pt", bufs=5,
                                          name=f"pt{tt}{h}{g}")
                        if debug and tt == 0 and h == 0:
                            nc.vector.memset(pt[:], 0.0)
                        if jc0 >= 4 * tt:
                            # band group: exact per-chunk exp + causal zeroing
                            # (regions outside [k*512+a, (k+1)*512) are never
                            # read downstream, so they stay unwritten)
                            for k, a in ((0, a0), (1, a1)):
                                lo, hi = k * 512 + a, (k + 1) * 512
                                nc.scalar.activation(
                                    pt[:, lo:hi], st[:, lo:hi],
                                    mybir.ActivationFunctionType.Exp)
                                nc.gpsimd.affine_select(
                                    out=pt[:, lo:hi], in_=pt[:, lo:hi],
                                    compare_op=mybir.AluOpType.is_ge,
                                    fill=0.0, base=0,
                                    pattern=[[1, 512 - a]],
                                    channel_multiplier=-1)
                        else:
                            nc.scalar.activation(
                                pt[:, 0:1024], st[:, 0:1024],
                                mybir.ActivationFunctionType.Exp)
                        if debug and tt == 0 and h == 0:
                            nc.sync.dma_start(out=dbg_pt[g], in_=pt[:])
                            if g == 0:
                                nc.sync.dma_start(out=dbg_kq[0],
                                                  in_=kt[(0, 0)][:])
                                nc.sync.dma_start(out=dbg_kq[1],
                                                  in_=qt[(0, 0)][:])
                                for _j in range(4):
                                    nc.sync.dma_start(out=dbg_v[_j],
                                                      in_=v[_j][:])
                        held = (tt, h, pt, ((jc0, a0), (jc1, a1)))
                    if pending_norm is not None:  # tt0: only 2 groups per head
                        emit_norm(*pending_norm)
                        pending_norm = None
                    pending_norm = (tt, h, ov_of[(tt, h)])
                while proj_units:
                    proj_units.pop(0)()
                while deferred:
                    deferred.pop(0)[0]()
                if w == TTn - 1:
                    if held is not None:
                        emit_avs(held)
                        held = None
                    if pending_norm is not None:
                        emit_norm(*pending_norm)
                        pending_norm = None
                    for cp in range(8):
                        emit_outproj_group(TTn - 1, cp)
                    emit_rs(TTn - 1)

    nc.compile()
    return nc


_NC_CACHE = {}


def _get_nc(with_rs: bool = True):
    key = bool(with_rs)
    if key not in _NC_CACHE:
        _NC_CACHE[key] = build_nc(with_rs)
    return _NC_CACHE[key]


def make_in_maps(x, Wq, Wk, Wv, Wo, bo):
    bf16 = ml_dtypes.bfloat16
    x = np.asarray(x, dtype=np.float32)
    Wq = np.asarray(Wq, dtype=np.float32)
    Wk = np.asarray(Wk, dtype=np.float32)
    Wv = np.asarray(Wv, dtype=np.float32)
    Wo = np.asarray(Wo, dtype=np.float32)
    bo = np.asarray(bo, dtype=np.float32)

    scale = np.float32(C) ** np.float32(-0.5)
    in_maps = []
    for c in range(N_CORES):
        b, hoff = c // 2, (c % 2) * HC
        heads = slice(hoff, hoff + HC)
        xT_c = np.ascontiguousarray(x[b].T).astype(bf16)             # [C, T]
        wq_c = np.ascontiguousarray(
            np.concatenate(list(Wq[heads] * scale), axis=1)).astype(bf16)
        wk_c = np.ascontiguousarray(
            np.concatenate(list(Wk[heads]), axis=1)).astype(bf16)
        wv_c = np.ascontiguousarray(
            np.concatenate(list(Wv[heads]), axis=1)).astype(bf16)
        wot_c = np.ascontiguousarray(
            Wo[:, hoff * D:(hoff + HC) * D].T).astype(bf16)          # [512, C]
        bo2_c = np.ascontiguousarray((bo / 2.0).reshape(8, 128).T)   # [128, 8]
        in_maps.append({
            "xT": xT_c, "wq": wq_c, "wk": wk_c, "wv": wv_c,
            "wot": wot_c, "bo2": bo2_c,
        })
    return in_maps


def kernel(x, Wq, Wk, Wv, Wo, bo):
    nc = _get_nc(with_rs=True)
    in_maps = make_in_maps(x, Wq, Wk, Wv, Wo, bo)
    # The axon-tunneled devices occasionally fail transiently
    # (NRT_EXEC_UNIT_UNRECOVERABLE / tunnel hangup); a retry recovers.
    last_err = None
    for _ in range(3):
        try:
            res = run_bass_kernel_spmd(nc, in_maps, list(range(N_CORES))).results
            break
        except Exception as e:  # noqa: BLE001
            last_err = e
            import time
            time.sleep(5)
    else:
        raise last_err

    out = np.empty((B, T, C), dtype=np.float32)
    for c in range(N_CORES):
        b, e = c // 2, c % 2
        yc = np.asarray(res[c]["y"], dtype=np.float32)  # [tt, c' slab, t]
        for tt in range(TTn):
            out[b, tt * 512:(tt + 1) * 512, e * 512:(e + 1) * 512] = yc[tt].T
    return out


# revision 43
# speedup vs baseline: 1.0595x; 1.0022x over previous
"""Multi-head causal attention (B=4, T=2048, C=1024, H=16, D=64) on 8 trn2 cores.

Sharding: tensor-parallel over heads within batch core-pairs.
  core c -> batch b = c//2, heads hoff..hoff+7 where hoff = (c%2)*8.

v2 design (all-bf16 dataflow; 377.8us baseline -> 284.2us):
  - Projections (Q^T/K^T per head-pair, V per key-chunk with a folded ones
    column) are software-pipelined INTO the attention loop as PE filler work:
    slab s+1's projection matmuls are paced between slab s's attention
    groups, because attention is ACT(exp)-bound while projections are pure
    PE. Inputs arrive as a handful of large batched strided DMAs (per-DMA
    queue overhead ~0.6us dominates small transfers).
  - Attention emission is organized in 4 windows whose head lists pull the
    first two heads of the next slab forward (their exps fill the ACT-idle
    tail of the previous window); within a head, the diagonal band groups
    run FIRST and off-band groups last, so the next head's scores never
    stall on a bunched-up chain of band exps.
  - Scores per head in S^T = [key, query] orientation, exp without
    max-subtraction (scores ~N(0, 0.25^2)), exact per-chunk exp ranges and
    per-chunk causal affine_selects (never touching unwritten PSUM).
  - AV in O-orientation: stationary = exp(S^T) chunk [128k, 128q], moving =
    V [128k, 65] (col 64 = ones -> softmax sums land in ov col 64). 65-row
    bf16 matmuls halve AV PE time vs the O^T orientation. PSUM zero-region
    note: start=True arms pending-zero for the whole 2KB region, so it is
    issued exactly once per head (first AV); later qq slices first-touch-
    replace and then accumulate.
  - Normalization: per-partition reciprocal + tensor_tensor multiply (queries
    are on partitions in O-layout), then O is transposed back to O^T via
    128-row matmuls against an identity for the output projection.
  - Output projection to partial y^T (+bo/2) in bf16; pairwise ReduceScatter
    (bf16 payload halves the 15us+bytes/40GBps collective cost; 4 t-slabs)
    sums partner partials; core even keeps c' 0:512, odd keeps 512:1024.
    The final slab's outproj pre-accumulates head-pairs 0..2 during the last
    head's slots so only the cl=3 matmul + one combine sit before the last
    (fully exposed) ReduceScatter.
Host reassembles the [B, T, C] f32 output by transposing/concatenating slabs.
"""

import numpy as np
import ml_dtypes

import concourse.bass as bass
import concourse.mybir as mybir
from concourse import bacc
from concourse.tile import TileContext
from concourse.bass_utils import run_bass_kernel_spmd

F32 = mybir.dt.float32
BF16 = mybir.dt.bfloat16

B, T, C = 4, 2048, 1024
H, D = 16, 64
HC = 8           # heads per core
NPAIR = HC // 2  # head pairs
CCn = C // 128   # 8 contraction chunks
TTn = T // 512   # 4 slabs of 512
JCn = T // 128   # 16 key chunks of 128
N_CORES = 8
RG = [[0, 1], [2, 3], [4, 5], [6, 7]]


def build_nc(with_rs: bool = True, debug: bool = False):
    nc = bacc.Bacc(None, target_bir_lowering=False)

    xT = nc.declare_dram_parameter("xT", [C, T], BF16, isOutput=False)
    wq = nc.declare_dram_parameter("wq", [C, 512], BF16, isOutput=False)
    wk = nc.declare_dram_parameter("wk", [C, 512], BF16, isOutput=False)
    wv = nc.declare_dram_parameter("wv", [C, 512], BF16, isOutput=False)
    wot = nc.declare_dram_parameter("wot", [512, C], BF16, isOutput=False)
    bo2 = nc.declare_dram_parameter("bo2", [128, 8], F32, isOutput=False)
    y = nc.declare_dram_parameter("y", [TTn, 512, 512], BF16, isOutput=True)
    if debug:
        dbg_kq = nc.declare_dram_parameter("dbg_kq", [2, 128, 512], BF16,
                                           isOutput=True)
        dbg_v = nc.declare_dram_parameter("dbg_v", [4, 128, 520], BF16,
                                          isOutput=True)
        dbg_pt = nc.declare_dram_parameter("dbg_pt", [2, 128, 1024], BF16,
                                           isOutput=True)
        dbg_ov = nc.declare_dram_parameter("dbg_ov", [128, 260], F32,
                                           isOutput=True)
        dbg_op = nc.declare_dram_parameter("dbg_op", [128, 512], BF16,
                                           isOutput=True)
        dbg_ot = nc.declare_dram_parameter("dbg_ot", [128, 512], BF16,
                                           isOutput=True)

    with TileContext(nc) as tc:
        with (
            tc.tile_pool(name="persist", bufs=1) as sb,
            tc.tile_pool(name="psum", bufs=1, space="PSUM") as psum,
            tc.tile_pool(name="dram", bufs=1, space="DRAM") as dram,
        ):
            # ---- persistent SBUF tiles (per-slab splits avoid false deps
            # between interleaved projection writes and attention reads) ----
            qt = {(p, s): sb.tile([128, 512], BF16, tag=f"qt{p}_{s}", name=f"qt{p}_{s}")
                  for p in range(NPAIR) for s in range(TTn)}
            kt = {(p, s): sb.tile([128, 512], BF16, tag=f"kt{p}_{s}", name=f"kt{p}_{s}")
                  for p in range(NPAIR) for s in range(TTn)}
            v = [sb.tile([128, 65 * HC], BF16, tag=f"v{j}", name=f"v{j}") for j in range(JCn)]
            ot = {(p, s): sb.tile([128, 512], BF16, tag=f"ot{p}_{s}", name=f"ot{p}_{s}")
                  for p in range(NPAIR) for s in range(TTn)}
            wqb = sb.tile([128, CCn * 512], BF16, tag="wqb", name="wqb")
            wkb = sb.tile([128, CCn * 512], BF16, tag="wkb", name="wkb")
            wvb = sb.tile([128, CCn * 512], BF16, tag="wvb", name="wvb")
            wob = sb.tile([128, 4 * C], BF16, tag="wob", name="wob")
            ones8 = sb.tile([128, HC], BF16, tag="ones8")
            ident = sb.tile([128, 128], BF16, tag="ident")
            bo_sb = sb.tile([128, 8], F32, tag="bo_sb")

            y_part = dram.tile([TTn, 1024, 512], BF16)
            rs_out = dram.tile([TTn, 512, 512], BF16)

            # ---- prologue DMAs: one batched strided transfer per tensor
            # (per-DMA queue overhead ~0.6us dominates small transfers) ----
            xtb_of = {}

            def issue_xts(s):
                i0 = s * 512
                t = sb.tile([128, CCn * 512], BF16, tag="xtb", bufs=2,
                            name=f"xtb{s}")
                nc.sync.dma_start(
                    out=t[:].rearrange("p (cc t) -> p cc t", cc=CCn),
                    in_=xT[:, i0:i0 + 512].rearrange(
                        "(cc p) t -> p cc t", cc=CCn))
                xtb_of[s] = t

            t0_ = sb.tile([128, CCn * 512], BF16, tag="xtb", bufs=2,
                          name="xtb0")
            xtb_of[0] = t0_
            for hh in range(4):
                cs = slice(hh * 2 * 512, (hh + 1) * 2 * 512)
                rs_ = slice(hh * 2 * 128, (hh + 1) * 2 * 128)
                nc.sync.dma_start(
                    out=wkb[:, cs].rearrange("p (cc j) -> p cc j", cc=2),
                    in_=wk[rs_, :].rearrange("(cc p) j -> p cc j", cc=2))
                nc.sync.dma_start(
                    out=t0_[:, cs].rearrange("p (cc t) -> p cc t", cc=2),
                    in_=xT[rs_, 0:512].rearrange("(cc p) t -> p cc t", cc=2))
                nc.sync.dma_start(
                    out=wqb[:, cs].rearrange("p (cc j) -> p cc j", cc=2),
                    in_=wq[rs_, :].rearrange("(cc p) j -> p cc j", cc=2))
            nc.sync.dma_start(
                out=wvb[:].rearrange("p (cc j) -> p cc j", cc=CCn),
                in_=wv[:].rearrange("(cc p) j -> p cc j", cc=CCn))
            nc.sync.dma_start(
                out=wob[:].rearrange("p (cl j) -> p cl j", cl=4),
                in_=wot[:].rearrange("(cl p) j -> p cl j", cl=4))
            nc.sync.dma_start(out=bo_sb[:], in_=bo2[:])
            nc.vector.memset(ones8[:], 1.0)
            # identity for O -> O^T transposes: memset 1, keep only the diagonal
            nc.vector.memset(ident[:], 1.0)
            nc.gpsimd.affine_select(
                out=ident[:], in_=ident[:], compare_op=mybir.AluOpType.is_ge,
                fill=0.0, base=0, pattern=[[1, 128]], channel_multiplier=-1)
            nc.gpsimd.affine_select(
                out=ident[:], in_=ident[:], compare_op=mybir.AluOpType.is_ge,
                fill=0.0, base=0, pattern=[[-1, 128]], channel_multiplier=1)

            # ---- projection emission units (filler work for the PE) ----
            # Each unit emits ~2-4 matmuls (~0.4us of PE). A group's PSUM tile
            # is allocated by its first unit; the last unit emits the copy out.
            def make_proj_units(s, defer_qk_pairs=()):
                units = []
                unit_chunks = []   # list of per-group unit lists, woven below
                deferred = []  # (unit, deadline_slot) for the NEXT slab
                xtb = xtb_of[s]

                def qk_group(wb, pair, dst_tile, defer_dl=None):
                    cell = {}
                    for cc0 in range(0, CCn, 2):
                        def u(cc0=cc0, wb=wb, pair=pair, cell=cell):
                            if cc0 == 0:
                                cell["ps"] = psum.tile([128, 512], F32,
                                                       tag="yps", bufs=2,
                                                       name="qkps")
                            ps = cell["ps"]
                            for cc in (cc0, cc0 + 1):
                                nc.tensor.matmul(
                                    ps[:],
                                    wb[:, cc * 512 + pair * 128:
                                       cc * 512 + pair * 128 + 128],
                                    xtb[:, cc * 512:(cc + 1) * 512],
                                    start=(cc == 0), stop=(cc == CCn - 1),
                                    skip_group_check=True)
                            if cc0 == CCn - 2:
                                nc.vector.tensor_copy(dst_tile[:], ps[:])
                        if defer_dl is None:
                            cur_chunk.append(u)
                        else:
                            deferred.append((u, defer_dl))

                ng = 2 * (s + 1)  # groups/head in the consuming slab
                for p in range(NPAIR):
                    cur_chunk = []
                    if p in defer_qk_pairs:
                        # consumed in the NEXT window, whose head list starts
                        # at h=2: head (s,2p) sits at slot ng*(2p-2)
                        dl = ng * (2 * p - 2) - 1
                        qk_group(wkb, p, kt[(p, s)], defer_dl=dl)
                        qk_group(wqb, p, qt[(p, s)], defer_dl=dl)
                    else:
                        qk_group(wkb, p, kt[(p, s)])
                        qk_group(wqb, p, qt[(p, s)])
                    unit_chunks.append(cur_chunk)

                def v_group(jc):
                    jl = jc * 128 - s * 512
                    cell = {}
                    for i in range(4):
                        def u(i=i, jc=jc, jl=jl, cell=cell):
                            if i == 0:
                                cell["ps"] = psum.tile([128, 512], F32,
                                                       tag="yps", bufs=2, name="vps")
                            ps = cell["ps"]
                            for m in range(4):
                                gm = i * 4 + m
                                g, cc = gm // 8, gm % 8
                                nc.tensor.matmul(
                                    ps[:, g * 256:(g + 1) * 256],
                                    xtb[:, cc * 512 + jl:cc * 512 + jl + 128],
                                    wvb[:, cc * 512 + g * 256:
                                        cc * 512 + g * 256 + 256],
                                    start=(cc == 0), stop=(cc == CCn - 1),
                                    skip_group_check=True)
                            if i == 3:
                                vv = v[jc][:].rearrange(
                                    "p (h e) -> p h e", h=HC, e=65)
                                nc.vector.tensor_copy(vv[:, :, 0:64], ps[:])
                                nc.vector.tensor_copy(vv[:, :, 64:65],
                                                      ones8[:])
                        cur_chunk.append(u)

                for jc in range(4 * s, 4 * s + 4):
                    cur_chunk = []
                    v_group(jc)
                    unit_chunks.append(cur_chunk)
                # weave: K0Q0, V0, V1, K1Q1, V2, V3, K2Q2, K3Q3 — V chunks
                # early enough that pulled-forward heads of slab s (processed
                # late in window s-1) see their v[] tiles written in time
                qks, vs = unit_chunks[:NPAIR], unit_chunks[NPAIR:]
                for i, chunk in enumerate([qks[0], vs[0], vs[1], qks[1],
                                           vs[2], vs[3], qks[2], qks[3]]):
                    units.extend(chunk)
                deferred.sort(key=lambda t: t[1])
                return units, deferred

            # interleave order inside PROJ(0) so tt0/h0 attention can start
            # as early as possible: K0,Q0,V0,V1 then the rest
            units0, _ = make_proj_units(0)
            # units0 layout: [K0(4), Q0(4), K1(4), Q1(4), ... V groups(4x4)]
            order0 = (units0[0:8] + units0[32:40] + units0[8:16]
                      + units0[40:48] + units0[16:32])
            for u in order0:
                u()
            deferred_next = []

            # ---- attention + interleaved projections ----
            pt_pool = sb
            held = None          # pending AV emission for the previous group
            pending_norm = None  # (tt, h, ov) awaiting recip+TT
            opair_cell = {}      # pair -> o_pair staging tile

            def emit_avs(hd):
                tt_, h_, pt_, a_of = hd
                ovt = ov_of[(tt_, h_)]
                for qq in range(4):
                    for idx, (jc, a) in enumerate(a_of):
                        if a <= qq * 128:
                            c0 = idx * 512 + qq * 128
                            # start only once per head: start=True arms a
                            # pending-zero over the whole 2KB PSUM zero
                            # region, so later qq slices first-touch-replace
                            # rather than re-arm (which would wipe earlier
                            # slices' partials on their next accumulate).
                            nc.tensor.matmul(
                                ovt[:, qq * 65:qq * 65 + 65],
                                pt_[:, c0:c0 + 128],
                                v[jc][:, h_ * 65:(h_ + 1) * 65],
                                start=(jc == 4 * tt_ and qq == 0),
                                stop=(jc == 4 * tt_ - 1) if tt_ >= 1
                                else (jc == qq),
                                skip_group_check=True)

            ov_of = {}

            def emit_norm(tt, h, ov):
                p, e = h // 2, h % 2
                if e == 0:
                    opair_cell[p] = sb.tile([128, 512], BF16, tag="opair",
                                            bufs=2, name=f"op{tt}{p}")
                opair = opair_cell[p]
                ovr = ov[:].rearrange("p (q o e) -> p q o e", q=4, o=1, e=65)
                rl = sb.tile([128, 4], F32, tag="rl", bufs=2, name="rl")
                nc.vector.reciprocal(
                    rl[:].rearrange("p (q o) -> p q o", q=4, o=1),
                    ov[:].rearrange("p (q e) -> p q e", q=4, e=65)[:, :, 64:65])
                opr = opair[:].rearrange("p (q hh e) -> p q hh e",
                                         q=4, hh=2, e=64)
                nc.vector.tensor_mul(
                    opr[:, :, e:e + 1, :], ovr[:, :, :, 0:64],
                    rl[:].rearrange("p (q o u) -> p q o u", q=4, o=1, u=1)
                    .broadcast_to((128, 4, 1, 64)))
                if debug and (tt, h) == (0, 0):
                    ov_stage = sb.tile([128, 260], F32, tag="dbgov",
                                       name="dbgov")
                    nc.vector.tensor_copy(ov_stage[:], ov[:])
                    nc.sync.dma_start(out=dbg_ov[:], in_=ov_stage[:])
                if e == 1:
                    # pair complete: transpose O -> O^T into ot[(p, tt)]
                    trp = psum.tile([128, 512], F32, tag="yps", bufs=2,
                                    name=f"tr{tt}{p}")
                    for qq in range(4):
                        nc.tensor.matmul(
                            trp[:, qq * 128:(qq + 1) * 128],
                            opair[:, qq * 128:(qq + 1) * 128],
                            ident[:], start=True, stop=True,
                            skip_group_check=True)
                    nc.vector.tensor_copy(ot[(p, tt)][:], trp[:])
                    if debug and (tt, h) == (0, 1):
                        nc.sync.dma_start(out=dbg_op[:], in_=opair[:])
                        nc.sync.dma_start(out=dbg_ot[:], in_=ot[(p, tt)][:])
                    del opair_cell[p]

            pg_tiles = {}

            def emit_partial_outproj(cp):
                # cl 0..2 partial sum for the final slab, staged to SBUF so
                # the epilogue only needs the cl=3 matmul + one combine
                yps = psum.tile([128, 512], F32, tag="yps", bufs=2,
                                name=f"pg{cp}")
                for cl in range(3):
                    nc.tensor.matmul(
                        yps[:], wob[:, cl * C + cp * 128:cl * C + cp * 128 + 128],
                        ot[(cl, TTn - 1)][:], start=(cl == 0), stop=(cl == 2),
                        skip_group_check=True)
                pg = sb.tile([128, 512], F32, tag=f"pg{cp}", name=f"pgs{cp}")
                nc.vector.tensor_copy(pg[:], yps[:])
                pg_tiles[cp] = pg

            def emit_outproj_group(tt, cp):
                yps = psum.tile([128, 512], F32, tag="yps", bufs=2,
                                name=f"yps{tt}{cp}")
                pg = pg_tiles.get(cp) if tt == TTn - 1 else None
                cl0 = 3 if pg is not None else 0
                for cl in range(cl0, 4):
                    nc.tensor.matmul(
                        yps[:], wob[:, cl * C + cp * 128:cl * C + cp * 128 + 128],
                        ot[(cl, tt)][:], start=(cl == cl0), stop=(cl == 3),
                        skip_group_check=True)
                ysb = sb.tile([128, 512], BF16, tag="ysb", bufs=8, name="ysb")
                if pg is not None:
                    nc.vector.scalar_tensor_tensor(
                        ysb[:], yps[:], bo_sb[:, cp:cp + 1], pg[:],
                        mybir.AluOpType.add, mybir.AluOpType.add)
                else:
                    nc.vector.tensor_scalar_add(ysb[:], yps[:],
                                                bo_sb[:, cp:cp + 1])
                nc.sync.dma_start(
                    out=y_part[tt, cp * 128:(cp + 1) * 128, :], in_=ysb[:])

            def emit_rs(tt):
                if with_rs:
                    nc.gpsimd.collective_compute(
                        "ReduceScatter", mybir.AluOpType.add,
                        replica_groups=RG,
                        ins=[y_part[tt]], outs=[rs_out[tt]])
                    nc.sync.dma_start(out=y[tt], in_=rs_out[tt])
                else:
                    nc.sync.dma_start(out=y[tt], in_=y_part[tt, 0:512, :])

            windows = [
                [(0, h) for h in range(HC)],
                [(1, h) for h in range(HC)] + [(2, 0), (2, 1)],
                [(2, h) for h in range(2, HC)] + [(3, 0), (3, 1)],
                [(3, h) for h in range(2, HC)],
            ]
            for w in range(TTn):
                head_list = windows[w]
                deferred = deferred_next  # deferred here from the prior make
                if w < TTn - 1:
                    issue_xts(w + 1)
                    proj_units, deferred_next = make_proj_units(
                        w + 1, defer_qk_pairs=(2, 3) if w == 2 else ())
                else:
                    proj_units = []
                    deferred_next = []
                slot_idx = [0]
                emitted = [0]
                total_slots = sum(2 * (t_ + 1) for t_, _ in head_list)
                total_def = len(deferred)

                def emit_filler():
                    si = slot_idx[0]
                    slot_idx[0] += 1
                    rem_slots = total_slots - si
                    if proj_units and rem_slots > 0:
                        n = -(-len(proj_units) // rem_slots)  # ceil
                        for _ in range(min(n, 8)):
                            if proj_units:
                                proj_units.pop(0)()
                    # deferred units: emit when due (deadline) or to keep
                    # proportional pace across the whole slab
                    target = (si + 1) * total_def // max(total_slots, 1)
                    while deferred and (deferred[0][1] <= si + 1
                                        or emitted[0] < target):
                        deferred.pop(0)[0]()
                        emitted[0] += 1

                for tt, h in head_list:
                    i0 = tt * 512
                    n_g = 2 * (tt + 1)
                    g_seq = [2 * tt, 2 * tt + 1] + list(range(2 * tt))
                    p, e = h // 2, h % 2
                    ov_of[(tt, h)] = psum.tile([128, 260], F32, tag="ovps",
                                               bufs=2, name=f"ov{tt}{h}")
                    for gi, g in enumerate(g_seq):
                        jc0, jc1 = 2 * g, 2 * g + 1
                        a0 = max(0, (jc0 - 4 * tt)) * 128
                        a1 = max(0, (jc1 - 4 * tt)) * 128
                        st = psum.tile([128, 1024], F32, tag="stps", bufs=2,
                                       name=f"st{tt}{h}{g}")
                        for k, (jc, a) in enumerate(((jc0, a0), (jc1, a1))):
                            nc.tensor.matmul(
                                st[:, k * 512 + a:(k + 1) * 512],
                                kt[(p, jc // 4)][e * 64:(e + 1) * 64,
                                                 (jc % 4) * 128:
                                                 (jc % 4) * 128 + 128],
                                qt[(p, tt)][e * 64:(e + 1) * 64, a:512],
                                start=True, stop=True, skip_group_check=True)
                        if held is not None:
                            emit_avs(held)
                            held = None
                        if gi == 0 and tt >= 1 and h in (1, 2, 3, 4):
                            for g2 in range(2):
                                emit_outproj_group(tt - 1, 2 * (h - 1) + g2)
                            if h == 4:
                                emit_rs(tt - 1)
                        if gi == 1 and pending_norm is not None:
                            emit_norm(*pending_norm)
                            pending_norm = None
                        if (tt == TTn - 1 and h == 7 and len(pg_tiles) < 8):
                            emit_partial_outproj(len(pg_tiles))
                        emit_filler()
                        pt = pt_pool.tile([128, 1024], BF16, tag="pt", bufs=5,
                                          name=f"pt{tt}{h}{g}")
                        if debug and tt == 0 and h == 0:
                            nc.vector.memset(pt[:], 0.0)
                        if jc0 >= 4 * tt:
                            # band group: exact per-chunk exp + causal zeroing
                            # (regions outside [k*512+a, (k+1)*512) are never
                            # read downstream, so they stay unwritten)
                            for k, a in ((0, a0), (1, a1)):
                                lo, hi = k * 512 + a, (k + 1) * 512
                                nc.scalar.activation(
                                    pt[:, lo:hi], st[:, lo:hi],
                                    mybir.ActivationFunctionType.Exp)
                                nc.gpsimd.affine_select(
                                    out=pt[:, lo:hi], in_=pt[:, lo:hi],
                                    compare_op=mybir.AluOpType.is_ge,
                                    fill=0.0, base=0,
                                    pattern=[[1, 512 - a]],
                                    channel_multiplier=-1)
                        else:
                            nc.scalar.activation(
                                pt[:, 0:1024], st[:, 0:1024],
                                mybir.ActivationFunctionType.Exp)
                        if debug and tt == 0 and h == 0:
                            nc.sync.dma_start(out=dbg_pt[g], in_=pt[:])
                            if g == 0:
                                nc.sync.dma_start(out=dbg_kq[0],
                                                  in_=kt[(0, 0)][:])
                                nc.sync.dma_start(out=dbg_kq[1],
                                                  in_=qt[(0, 0)][:])
                                for _j in range(4):
                                    nc.sync.dma_start(out=dbg_v[_j],
                                                      in_=v[_j][:])
                        held = (tt, h, pt, ((jc0, a0), (jc1, a1)))
                    if pending_norm is not None:  # tt0: only 2 groups per head
                        emit_norm(*pending_norm)
                        pending_norm = None
                    pending_norm = (tt, h, ov_of[(tt, h)])
                while proj_units:
                    proj_units.pop(0)()
                while deferred:
                    deferred.pop(0)[0]()
                if w == TTn - 1:
                    if held is not None:
                        emit_avs(held)
                        held = None
                    if pending_norm is not None:
                        emit_norm(*pending_norm)
                        pending_norm = None
                    for cp in range(8):
                        emit_outproj_group(TTn - 1, cp)
                    emit_rs(TTn - 1)

    nc.compile()
    return nc


_NC_CACHE = {}


def _get_nc(with_rs: bool = True):
    key = bool(with_rs)
    if key not in _NC_CACHE:
        _NC_CACHE[key] = build_nc(with_rs)
    return _NC_CACHE[key]


def make_in_maps(x, Wq, Wk, Wv, Wo, bo):
    bf16 = ml_dtypes.bfloat16
    x = np.asarray(x, dtype=np.float32)
    Wq = np.asarray(Wq, dtype=np.float32)
    Wk = np.asarray(Wk, dtype=np.float32)
    Wv = np.asarray(Wv, dtype=np.float32)
    Wo = np.asarray(Wo, dtype=np.float32)
    bo = np.asarray(bo, dtype=np.float32)

    scale = np.float32(C) ** np.float32(-0.5)
    in_maps = []
    for c in range(N_CORES):
        b, hoff = c // 2, (c % 2) * HC
        heads = slice(hoff, hoff + HC)
        xT_c = np.ascontiguousarray(x[b].T).astype(bf16)             # [C, T]
        wq_c = np.ascontiguousarray(
            np.concatenate(list(Wq[heads] * scale), axis=1)).astype(bf16)
        wk_c = np.ascontiguousarray(
            np.concatenate(list(Wk[heads]), axis=1)).astype(bf16)
        wv_c = np.ascontiguousarray(
            np.concatenate(list(Wv[heads]), axis=1)).astype(bf16)
        wot_c = np.ascontiguousarray(
            Wo[:, hoff * D:(hoff + HC) * D].T).astype(bf16)          # [512, C]
        bo2_c = np.ascontiguousarray((bo / 2.0).reshape(8, 128).T)   # [128, 8]
        in_maps.append({
            "xT": xT_c, "wq": wq_c, "wk": wk_c, "wv": wv_c,
            "wot": wot_c, "bo2": bo2_c,
        })
    return in_maps


def kernel(x, Wq, Wk, Wv, Wo, bo):
    nc = _get_nc(with_rs=True)
    in_maps = make_in_maps(x, Wq, Wk, Wv, Wo, bo)
    # The axon-tunneled devices occasionally fail transiently
    # (NRT_EXEC_UNIT_UNRECOVERABLE / tunnel hangup); a retry recovers.
    last_err = None
    for _ in range(3):
        try:
            res = run_bass_kernel_spmd(nc, in_maps, list(range(N_CORES))).results
            break
        except Exception as e:  # noqa: BLE001
            last_err = e
            import time
            time.sleep(5)
    else:
        raise last_err

    out = np.empty((B, T, C), dtype=np.float32)
    for c in range(N_CORES):
        b, e = c // 2, c % 2
        yc = np.asarray(res[c]["y"], dtype=np.float32)  # [tt, c' slab, t]
        for tt in range(TTn):
            out[b, tt * 512:(tt + 1) * 512, e * 512:(e + 1) * 512] = yc[tt].T
    return out


# revision 45
# speedup vs baseline: 1.0647x; 1.0049x over previous
"""Multi-head causal attention (B=4, T=2048, C=1024, H=16, D=64) on 8 trn2 cores.

Sharding: tensor-parallel over heads within batch core-pairs.
  core c -> batch b = c//2, heads hoff..hoff+7 where hoff = (c%2)*8.

v2 design (all-bf16 dataflow; 377.8us baseline -> 284.2us):
  - Projections (Q^T/K^T per head-pair, V per key-chunk with a folded ones
    column) are software-pipelined INTO the attention loop as PE filler work:
    slab s+1's projection matmuls are paced between slab s's attention
    groups, because attention is ACT(exp)-bound while projections are pure
    PE. Inputs arrive as a handful of large batched strided DMAs (per-DMA
    queue overhead ~0.6us dominates small transfers).
  - Attention emission is organized in 4 windows whose head lists pull the
    first two heads of the next slab forward (their exps fill the ACT-idle
    tail of the previous window); within a head, the diagonal band groups
    run FIRST and off-band groups last, so the next head's scores never
    stall on a bunched-up chain of band exps.
  - Scores per head in S^T = [key, query] orientation, exp without
    max-subtraction (scores ~N(0, 0.25^2)), exact per-chunk exp ranges and
    per-chunk causal affine_selects (never touching unwritten PSUM).
  - AV in O-orientation: stationary = exp(S^T) chunk [128k, 128q], moving =
    V [128k, 65] (col 64 = ones -> softmax sums land in ov col 64). 65-row
    bf16 matmuls halve AV PE time vs the O^T orientation. PSUM zero-region
    note: start=True arms pending-zero for the whole 2KB region, so it is
    issued exactly once per head (first AV); later qq slices first-touch-
    replace and then accumulate.
  - Normalization: per-partition reciprocal + tensor_tensor multiply (queries
    are on partitions in O-layout), then O is transposed back to O^T via
    128-row matmuls against an identity for the output projection.
  - Output projection to partial y^T (+bo/2) in bf16; pairwise ReduceScatter
    (bf16 payload halves the 15us+bytes/40GBps collective cost; 4 t-slabs)
    sums partner partials; core even keeps c' 0:512, odd keeps 512:1024.
    The final slab's outproj pre-accumulates head-pairs 0..2 during the last
    head's slots so only the cl=3 matmul + one combine sit before the last
    (fully exposed) ReduceScatter.
Host reassembles the [B, T, C] f32 output by transposing/concatenating slabs.
"""

import numpy as np
import ml_dtypes

import concourse.bass as bass
import concourse.mybir as mybir
from concourse import bacc
from concourse.tile import TileContext
from concourse.bass_utils import run_bass_kernel_spmd

F32 = mybir.dt.float32
BF16 = mybir.dt.bfloat16

B, T, C = 4, 2048, 1024
H, D = 16, 64
HC = 8           # heads per core
NPAIR = HC // 2  # head pairs
CCn = C // 128   # 8 contraction chunks
TTn = T // 512   # 4 slabs of 512
JCn = T // 128   # 16 key chunks of 128
N_CORES = 8
RG = [[0, 1], [2, 3], [4, 5], [6, 7]]


def build_nc(with_rs: bool = True, debug: bool = False):
    nc = bacc.Bacc(None, target_bir_lowering=False)

    xT = nc.declare_dram_parameter("xT", [C, T], BF16, isOutput=False)
    wq = nc.declare_dram_parameter("wq", [C, 512], BF16, isOutput=False)
    wk = nc.declare_dram_parameter("wk", [C, 512], BF16, isOutput=False)
    wv = nc.declare_dram_parameter("wv", [C, 512], BF16, isOutput=False)
    wot = nc.declare_dram_parameter("wot", [512, C], BF16, isOutput=False)
    bo2 = nc.declare_dram_parameter("bo2", [128, 8], F32, isOutput=False)
    y = nc.declare_dram_parameter("y", [TTn, 512, 512], BF16, isOutput=True)
    if debug:
        dbg_kq = nc.declare_dram_parameter("dbg_kq", [2, 128, 512], BF16,
                                           isOutput=True)
        dbg_v = nc.declare_dram_parameter("dbg_v", [4, 128, 520], BF16,
                                          isOutput=True)
        dbg_pt = nc.declare_dram_parameter("dbg_pt", [2, 128, 1024], BF16,
                                           isOutput=True)
        dbg_ov = nc.declare_dram_parameter("dbg_ov", [128, 260], F32,
                                           isOutput=True)
        dbg_op = nc.declare_dram_parameter("dbg_op", [128, 512], BF16,
                                           isOutput=True)
        dbg_ot = nc.declare_dram_parameter("dbg_ot", [128, 512], BF16,
                                           isOutput=True)

    with TileContext(nc) as tc:
        with (
            tc.tile_pool(name="persist", bufs=1) as sb,
            tc.tile_pool(name="psum", bufs=1, space="PSUM") as psum,
            tc.tile_pool(name="dram", bufs=1, space="DRAM") as dram,
        ):
            # ---- persistent SBUF tiles (per-slab splits avoid false deps
            # between interleaved projection writes and attention reads) ----
            qt = {(p, s): sb.tile([128, 512], BF16, tag=f"qt{p}_{s}", name=f"qt{p}_{s}")
                  for p in range(NPAIR) for s in range(TTn)}
            kt = {(p, s): sb.tile([128, 512], BF16, tag=f"kt{p}_{s}", name=f"kt{p}_{s}")
                  for p in range(NPAIR) for s in range(TTn)}
            v = [sb.tile([128, 65 * HC], BF16, tag=f"v{j}", name=f"v{j}") for j in range(JCn)]
            ot = {(p, s): sb.tile([128, 512], BF16, tag=f"ot{p}_{s}", name=f"ot{p}_{s}")
                  for p in range(NPAIR) for s in range(TTn)}
            wqb = sb.tile([128, CCn * 512], BF16, tag="wqb", name="wqb")
            wkb = sb.tile([128, CCn * 512], BF16, tag="wkb", name="wkb")
            wvb = sb.tile([128, CCn * 512], BF16, tag="wvb", name="wvb")
            wob = sb.tile([128, 4 * C], BF16, tag="wob", name="wob")
            ones8 = sb.tile([128, HC], BF16, tag="ones8")
            ident = sb.tile([128, 128], BF16, tag="ident")
            bo_sb = sb.tile([128, 8], F32, tag="bo_sb")

            y_part = dram.tile([TTn, 1024, 512], BF16)
            rs_out = dram.tile([TTn, 512, 512], BF16)

            # ---- prologue DMAs: one batched strided transfer per tensor
            # (per-DMA queue overhead ~0.6us dominates small transfers) ----
            xtb_of = {}

            def issue_xts(s):
                i0 = s * 512
                t = sb.tile([128, CCn * 512], BF16, tag="xtb", bufs=2,
                            name=f"xtb{s}")
                nc.sync.dma_start(
                    out=t[:].rearrange("p (cc t) -> p cc t", cc=CCn),
                    in_=xT[:, i0:i0 + 512].rearrange(
                        "(cc p) t -> p cc t", cc=CCn))
                xtb_of[s] = t

            t0_ = sb.tile([128, CCn * 512], BF16, tag="xtb", bufs=2,
                          name="xtb0")
            xtb_of[0] = t0_
            for hh in range(4):
                cs = slice(hh * 2 * 512, (hh + 1) * 2 * 512)
                rs_ = slice(hh * 2 * 128, (hh + 1) * 2 * 128)
                nc.sync.dma_start(
                    out=wkb[:, cs].rearrange("p (cc j) -> p cc j", cc=2),
                    in_=wk[rs_, :].rearrange("(cc p) j -> p cc j", cc=2))
                nc.sync.dma_start(
                    out=t0_[:, cs].rearrange("p (cc t) -> p cc t", cc=2),
                    in_=xT[rs_, 0:512].rearrange("(cc p) t -> p cc t", cc=2))
                nc.sync.dma_start(
                    out=wqb[:, cs].rearrange("p (cc j) -> p cc j", cc=2),
                    in_=wq[rs_, :].rearrange("(cc p) j -> p cc j", cc=2))
            nc.sync.dma_start(
                out=wvb[:].rearrange("p (cc j) -> p cc j", cc=CCn),
                in_=wv[:].rearrange("(cc p) j -> p cc j", cc=CCn))
            nc.sync.dma_start(
                out=wob[:].rearrange("p (cl j) -> p cl j", cl=4),
                in_=wot[:].rearrange("(cl p) j -> p cl j", cl=4))
            nc.sync.dma_start(out=bo_sb[:], in_=bo2[:])
            nc.vector.memset(ones8[:], 1.0)
            # identity for O -> O^T transposes: memset 1, keep only the diagonal
            nc.vector.memset(ident[:], 1.0)
            nc.gpsimd.affine_select(
                out=ident[:], in_=ident[:], compare_op=mybir.AluOpType.is_ge,
                fill=0.0, base=0, pattern=[[1, 128]], channel_multiplier=-1)
            nc.gpsimd.affine_select(
                out=ident[:], in_=ident[:], compare_op=mybir.AluOpType.is_ge,
                fill=0.0, base=0, pattern=[[-1, 128]], channel_multiplier=1)

            # ---- projection emission units (filler work for the PE) ----
            # Each unit emits ~2-4 matmuls (~0.4us of PE). A group's PSUM tile
            # is allocated by its first unit; the last unit emits the copy out.
            def make_proj_units(s, defer_qk_pairs=()):
                units = []
                unit_chunks = []   # list of per-group unit lists, woven below
                deferred = []  # (unit, deadline_slot) for the NEXT slab
                xtb = xtb_of[s]

                def qk_group(wb, pair, dst_tile, defer_dl=None):
                    cell = {}
                    for cc0 in range(0, CCn, 2):
                        def u(cc0=cc0, wb=wb, pair=pair, cell=cell):
                            if cc0 == 0:
                                cell["ps"] = psum.tile([128, 512], F32,
                                                       tag="yps", bufs=2,
                                                       name="qkps")
                            ps = cell["ps"]
                            for cc in (cc0, cc0 + 1):
                                nc.tensor.matmul(
                                    ps[:],
                                    wb[:, cc * 512 + pair * 128:
                                       cc * 512 + pair * 128 + 128],
                                    xtb[:, cc * 512:(cc + 1) * 512],
                                    start=(cc == 0), stop=(cc == CCn - 1),
                                    skip_group_check=True)
                            if cc0 == CCn - 2:
                                nc.vector.tensor_copy(dst_tile[:], ps[:])
                        if defer_dl is None:
                            cur_chunk.append(u)
                        else:
                            deferred.append((u, defer_dl))

                ng = 2 * (s + 1)  # groups/head in the consuming slab
                for p in range(NPAIR):
                    cur_chunk = []
                    if p in defer_qk_pairs:
                        # consumed in the NEXT window, whose head list starts
                        # at h=2: head (s,2p) sits at slot ng*(2p-2)
                        dl = ng * (2 * p - 2) - 1
                        qk_group(wkb, p, kt[(p, s)], defer_dl=dl)
                        qk_group(wqb, p, qt[(p, s)], defer_dl=dl)
                    else:
                        qk_group(wkb, p, kt[(p, s)])
                        qk_group(wqb, p, qt[(p, s)])
                    unit_chunks.append(cur_chunk)

                def v_group(jc):
                    jl = jc * 128 - s * 512
                    cell = {}
                    for i in range(4):
                        def u(i=i, jc=jc, jl=jl, cell=cell):
                            if i == 0:
                                cell["ps"] = psum.tile([128, 512], F32,
                                                       tag="yps", bufs=2, name="vps")
                            ps = cell["ps"]
                            for m in range(4):
                                gm = i * 4 + m
                                g, cc = gm // 8, gm % 8
                                nc.tensor.matmul(
                                    ps[:, g * 256:(g + 1) * 256],
                                    xtb[:, cc * 512 + jl:cc * 512 + jl + 128],
                                    wvb[:, cc * 512 + g * 256:
                                        cc * 512 + g * 256 + 256],
                                    start=(cc == 0), stop=(cc == CCn - 1),
                                    skip_group_check=True)
                            if i == 3:
                                vv = v[jc][:].rearrange(
                                    "p (h e) -> p h e", h=HC, e=65)
                                nc.vector.tensor_copy(vv[:, :, 0:64], ps[:])
                                nc.vector.tensor_copy(vv[:, :, 64:65],
                                                      ones8[:])
                        cur_chunk.append(u)

                for jc in range(4 * s, 4 * s + 4):
                    cur_chunk = []
                    v_group(jc)
                    unit_chunks.append(cur_chunk)
                # weave: K0Q0, V0, V1, K1Q1, V2, V3, K2Q2, K3Q3 — V chunks
                # early enough that pulled-forward heads of slab s (processed
                # late in window s-1) see their v[] tiles written in time
                qks, vs = unit_chunks[:NPAIR], unit_chunks[NPAIR:]
                for i, chunk in enumerate([qks[0], vs[0], vs[1], qks[1],
                                           vs[2], vs[3], qks[2], qks[3]]):
                    units.extend(chunk)
                deferred.sort(key=lambda t: t[1])
                return units, deferred

            # interleave order inside PROJ(0) so tt0/h0 attention can start
            # as early as possible: K0,Q0,V0,V1 then the rest
            units0, _ = make_proj_units(0)
            # units0 layout: [K0(4), Q0(4), K1(4), Q1(4), ... V groups(4x4)]
            order0 = (units0[0:8] + units0[32:40] + units0[8:16]
                      + units0[40:48] + units0[16:32])
            for u in order0:
                u()
            deferred_next = []

            # ---- attention + interleaved projections ----
            pt_pool = sb
            held = None          # pending AV emission for the previous group
            pending_norm = None  # (tt, h, ov) awaiting recip+TT
            opair_cell = {}      # pair -> o_pair staging tile

            def emit_avs(hd):
                tt_, h_, pt_, a_of = hd
                ovt = ov_of[(tt_, h_)]
                for qq in range(4):
                    for idx, (jc, a) in enumerate(a_of):
                        if a <= qq * 128:
                            c0 = idx * 512 + qq * 128
                            # start only once per head: start=True arms a
                            # pending-zero over the whole 2KB PSUM zero
                            # region, so later qq slices first-touch-replace
                            # rather than re-arm (which would wipe earlier
                            # slices' partials on their next accumulate).
                            nc.tensor.matmul(
                                ovt[:, qq * 65:qq * 65 + 65],
                                pt_[:, c0:c0 + 128],
                                v[jc][:, h_ * 65:(h_ + 1) * 65],
                                start=(jc == 4 * tt_ and qq == 0),
                                stop=(jc == 4 * tt_ - 1) if tt_ >= 1
                                else (jc == qq),
                                skip_group_check=True)

            ov_of = {}

            def emit_norm(tt, h, ov):
                p, e = h // 2, h % 2
                if e == 0:
                    opair_cell[p] = sb.tile([128, 512], BF16, tag="opair",
                                            bufs=2, name=f"op{tt}{p}")
                opair = opair_cell[p]
                ovr = ov[:].rearrange("p (q o e) -> p q o e", q=4, o=1, e=65)
                rl = sb.tile([128, 4], F32, tag="rl", bufs=2, name="rl")
                nc.vector.reciprocal(
                    rl[:].rearrange("p (q o) -> p q o", q=4, o=1),
                    ov[:].rearrange("p (q e) -> p q e", q=4, e=65)[:, :, 64:65])
                opr = opair[:].rearrange("p (q hh e) -> p q hh e",
                                         q=4, hh=2, e=64)
                nc.vector.tensor_mul(
                    opr[:, :, e:e + 1, :], ovr[:, :, :, 0:64],
                    rl[:].rearrange("p (q o u) -> p q o u", q=4, o=1, u=1)
                    .broadcast_to((128, 4, 1, 64)))
                if debug and (tt, h) == (0, 0):
                    ov_stage = sb.tile([128, 260], F32, tag="dbgov",
                                       name="dbgov")
                    nc.vector.tensor_copy(ov_stage[:], ov[:])
                    nc.sync.dma_start(out=dbg_ov[:], in_=ov_stage[:])
                if e == 1:
                    # pair complete: transpose O -> O^T into ot[(p, tt)]
                    trp = psum.tile([128, 512], F32, tag="yps", bufs=2,
                                    name=f"tr{tt}{p}")
                    for qq in range(4):
                        nc.tensor.matmul(
                            trp[:, qq * 128:(qq + 1) * 128],
                            opair[:, qq * 128:(qq + 1) * 128],
                            ident[:], start=True, stop=True,
                            skip_group_check=True)
                    nc.vector.tensor_copy(ot[(p, tt)][:], trp[:])
                    if debug and (tt, h) == (0, 1):
                        nc.sync.dma_start(out=dbg_op[:], in_=opair[:])
                        nc.sync.dma_start(out=dbg_ot[:], in_=ot[(p, tt)][:])
                    del opair_cell[p]

            pg_tiles = {}

            def emit_partial_outproj(cp):
                # cl 0..2 partial sum for the final slab, staged to SBUF so
                # the epilogue only needs the cl=3 matmul + one combine
                yps = psum.tile([128, 512], F32, tag="yps", bufs=2,
                                name=f"pg{cp}")
                for cl in range(3):
                    nc.tensor.matmul(
                        yps[:], wob[:, cl * C + cp * 128:cl * C + cp * 128 + 128],
                        ot[(cl, TTn - 1)][:], start=(cl == 0), stop=(cl == 2),
                        skip_group_check=True)
                pg = sb.tile([128, 512], F32, tag=f"pg{cp}", name=f"pgs{cp}")
                nc.vector.tensor_copy(pg[:], yps[:])
                pg_tiles[cp] = pg

            def emit_outproj_group(tt, cp):
                yps = psum.tile([128, 512], F32, tag="yps", bufs=2,
                                name=f"yps{tt}{cp}")
                pg = pg_tiles.get(cp) if tt == TTn - 1 else None
                cl0 = 3 if pg is not None else 0
                for cl in range(cl0, 4):
                    nc.tensor.matmul(
                        yps[:], wob[:, cl * C + cp * 128:cl * C + cp * 128 + 128],
                        ot[(cl, tt)][:], start=(cl == cl0), stop=(cl == 3),
                        skip_group_check=True)
                ysb = sb.tile([128, 512], BF16, tag="ysb", bufs=8, name="ysb")
                if pg is not None:
                    nc.vector.scalar_tensor_tensor(
                        ysb[:], yps[:], bo_sb[:, cp:cp + 1], pg[:],
                        mybir.AluOpType.add, mybir.AluOpType.add)
                else:
                    nc.vector.tensor_scalar_add(ysb[:], yps[:],
                                                bo_sb[:, cp:cp + 1])
                nc.sync.dma_start(
                    out=y_part[tt, cp * 128:(cp + 1) * 128, :], in_=ysb[:])

            def emit_rs(tt):
                if with_rs:
                    nc.gpsimd.collective_compute(
                        "ReduceScatter", mybir.AluOpType.add,
                        replica_groups=RG,
                        ins=[y_part[tt]], outs=[rs_out[tt]])
                    nc.sync.dma_start(out=y[tt], in_=rs_out[tt])
                else:
                    nc.sync.dma_start(out=y[tt], in_=y_part[tt, 0:512, :])

            windows = [
                [(0, h) for h in range(HC)],
                [(1, h) for h in range(HC)] + [(2, 0), (2, 1)],
                [(2, h) for h in range(2, HC)] + [(3, 0), (3, 1)],
                [(3, h) for h in range(2, HC)],
            ]
            for w in range(TTn):
                head_list = windows[w]
                deferred = deferred_next  # deferred here from the prior make
                if w < TTn - 1:
                    issue_xts(w + 1)
                    proj_units, deferred_next = make_proj_units(
                        w + 1, defer_qk_pairs=(2, 3) if w == 2 else ())
                else:
                    proj_units = []
                    deferred_next = []
                slot_idx = [0]
                emitted = [0]
                total_slots = sum(2 * (t_ + 1) for t_, _ in head_list)
                total_def = len(deferred)

                def emit_filler():
                    si = slot_idx[0]
                    slot_idx[0] += 1
                    rem_slots = total_slots - si
                    if proj_units and rem_slots > 0:
                        n = -(-len(proj_units) // rem_slots)  # ceil
                        for _ in range(min(n, 8)):
                            if proj_units:
                                proj_units.pop(0)()
                    # deferred units: emit when due (deadline) or to keep
                    # proportional pace across the whole slab
                    target = (si + 1) * total_def // max(total_slots, 1)
                    while deferred and (deferred[0][1] <= si + 1
                                        or emitted[0] < target):
                        deferred.pop(0)[0]()
                        emitted[0] += 1

                for tt, h in head_list:
                    i0 = tt * 512
                    n_g = 2 * (tt + 1)
                    g_seq = [2 * tt, 2 * tt + 1] + list(range(2 * tt))
                    p, e = h // 2, h % 2
                    ov_of[(tt, h)] = psum.tile([128, 260], F32, tag="ovps",
                                               bufs=2, name=f"ov{tt}{h}")
                    for gi, g in enumerate(g_seq):
                        jc0, jc1 = 2 * g, 2 * g + 1
                        a0 = max(0, (jc0 - 4 * tt)) * 128
                        a1 = max(0, (jc1 - 4 * tt)) * 128
                        st = psum.tile([128, 1024], F32, tag="stps", bufs=2,
                                       name=f"st{tt}{h}{g}")
                        for k, (jc, a) in enumerate(((jc0, a0), (jc1, a1))):
                            nc.tensor.matmul(
                                st[:, k * 512 + a:(k + 1) * 512],
                                kt[(p, jc // 4)][e * 64:(e + 1) * 64,
                                                 (jc % 4) * 128:
                                                 (jc % 4) * 128 + 128],
                                qt[(p, tt)][e * 64:(e + 1) * 64, a:512],
                                start=True, stop=True, skip_group_check=True)
                        if held is not None:
                            emit_avs(held)
                            held = None
                        if gi == 0 and tt >= 1 and h in (1, 2, 3, 4):
                            for g2 in range(2):
                                emit_outproj_group(tt - 1, 2 * (h - 1) + g2)
                            if h == 4:
                                emit_rs(tt - 1)
                        if gi == 1 and pending_norm is not None:
                            emit_norm(*pending_norm)
                            pending_norm = None
                        if (tt == TTn - 1 and h == 7 and len(pg_tiles) < 8):
                            emit_partial_outproj(len(pg_tiles))
                        emit_filler()
                        pt = pt_pool.tile([128, 1024], BF16, tag="pt", bufs=8,
                                          name=f"pt{tt}{h}{g}")
                        if debug and tt == 0 and h == 0:
                            nc.vector.memset(pt[:], 0.0)
                        if jc0 >= 4 * tt:
                            # band group: exact per-chunk exp + causal zeroing
                            # (regions outside [k*512+a, (k+1)*512) are never
                            # read downstream, so they stay unwritten)
                            for k, a in ((0, a0), (1, a1)):
                                lo, hi = k * 512 + a, (k + 1) * 512
                                nc.scalar.activation(
                                    pt[:, lo:hi], st[:, lo:hi],
                                    mybir.ActivationFunctionType.Exp)
                                nc.gpsimd.affine_select(
                                    out=pt[:, lo:hi], in_=pt[:, lo:hi],
                                    compare_op=mybir.AluOpType.is_ge,
                                    fill=0.0, base=0,
                                    pattern=[[1, 512 - a]],
                                    channel_multiplier=-1)
                        else:
                            nc.scalar.activation(
                                pt[:, 0:1024], st[:, 0:1024],
                                mybir.ActivationFunctionType.Exp)
                        if debug and tt == 0 and h == 0:
                            nc.sync.dma_start(out=dbg_pt[g], in_=pt[:])
                            if g == 0:
                                nc.sync.dma_start(out=dbg_kq[0],
                                                  in_=kt[(0, 0)][:])
                                nc.sync.dma_start(out=dbg_kq[1],
                                                  in_=qt[(0, 0)][:])
                                for _j in range(4):
                                    nc.sync.dma_start(out=dbg_v[_j],
                                                      in_=v[_j][:])
                        held = (tt, h, pt, ((jc0, a0), (jc1, a1)))
                    if pending_norm is not None:  # tt0: only 2 groups per head
                        emit_norm(*pending_norm)
                        pending_norm = None
                    pending_norm = (tt, h, ov_of[(tt, h)])
                while proj_units:
                    proj_units.pop(0)()
                while deferred:
                    deferred.pop(0)[0]()
                if w == TTn - 1:
                    if held is not None:
                        emit_avs(held)
                        held = None
                    if pending_norm is not None:
                        emit_norm(*pending_norm)
                        pending_norm = None
                    for cp in range(8):
                        emit_outproj_group(TTn - 1, cp)
                    emit_rs(TTn - 1)

    nc.compile()
    return nc


_NC_CACHE = {}


def _get_nc(with_rs: bool = True):
    key = bool(with_rs)
    if key not in _NC_CACHE:
        _NC_CACHE[key] = build_nc(with_rs)
    return _NC_CACHE[key]


def make_in_maps(x, Wq, Wk, Wv, Wo, bo):
    bf16 = ml_dtypes.bfloat16
    x = np.asarray(x, dtype=np.float32)
    Wq = np.asarray(Wq, dtype=np.float32)
    Wk = np.asarray(Wk, dtype=np.float32)
    Wv = np.asarray(Wv, dtype=np.float32)
    Wo = np.asarray(Wo, dtype=np.float32)
    bo = np.asarray(bo, dtype=np.float32)

    scale = np.float32(C) ** np.float32(-0.5)
    in_maps = []
    for c in range(N_CORES):
        b, hoff = c // 2, (c % 2) * HC
        heads = slice(hoff, hoff + HC)
        xT_c = np.ascontiguousarray(x[b].T).astype(bf16)             # [C, T]
        wq_c = np.ascontiguousarray(
            np.concatenate(list(Wq[heads] * scale), axis=1)).astype(bf16)
        wk_c = np.ascontiguousarray(
            np.concatenate(list(Wk[heads]), axis=1)).astype(bf16)
        wv_c = np.ascontiguousarray(
            np.concatenate(list(Wv[heads]), axis=1)).astype(bf16)
        wot_c = np.ascontiguousarray(
            Wo[:, hoff * D:(hoff + HC) * D].T).astype(bf16)          # [512, C]
        bo2_c = np.ascontiguousarray((bo / 2.0).reshape(8, 128).T)   # [128, 8]
        in_maps.append({
            "xT": xT_c, "wq": wq_c, "wk": wk_c, "wv": wv_c,
            "wot": wot_c, "bo2": bo2_c,
        })
    return in_maps


def kernel(x, Wq, Wk, Wv, Wo, bo):
    nc = _get_nc(with_rs=True)
    in_maps = make_in_maps(x, Wq, Wk, Wv, Wo, bo)
    # The axon-tunneled devices occasionally fail transiently
    # (NRT_EXEC_UNIT_UNRECOVERABLE / tunnel hangup); a retry recovers.
    last_err = None
    for _ in range(3):
        try:
            res = run_bass_kernel_spmd(nc, in_maps, list(range(N_CORES))).results
            break
        except Exception as e:  # noqa: BLE001
            last_err = e
            import time
            time.sleep(5)
    else:
        raise last_err

    out = np.empty((B, T, C), dtype=np.float32)
    for c in range(N_CORES):
        b, e = c // 2, c % 2
        yc = np.asarray(res[c]["y"], dtype=np.float32)  # [tt, c' slab, t]
        for tt in range(TTn):
            out[b, tt * 512:(tt + 1) * 512, e * 512:(e + 1) * 512] = yc[tt].T
    return out


# revision 50
# speedup vs baseline: 1.0923x; 1.0259x over previous
"""Multi-head causal attention (B=4, T=2048, C=1024, H=16, D=64) on 8 trn2 cores.

Sharding: tensor-parallel over heads within batch core-pairs.
  core c -> batch b = c//2, heads hoff..hoff+7 where hoff = (c%2)*8.

v2 design (all-bf16 dataflow; 377.8us baseline -> 282.2us):
  - Projections (Q^T/K^T per head-pair, V per key-chunk with a folded ones
    column) are software-pipelined INTO the attention loop as PE filler work:
    slab s+1's projection matmuls are paced between slab s's attention
    groups, because attention is ACT(exp)-bound while projections are pure
    PE. Inputs arrive as a handful of large batched strided DMAs (per-DMA
    queue overhead ~0.6us dominates small transfers).
  - Attention emission is organized in 4 windows whose head lists pull the
    first two heads of the next slab forward (their exps fill the ACT-idle
    tail of the previous window); within a head, the diagonal band groups
    run FIRST and off-band groups last, so the next head's scores never
    stall on a bunched-up chain of band exps.
  - Scores per head in S^T = [key, query] orientation, exp without
    max-subtraction (scores ~N(0, 0.25^2)), exact per-chunk exp ranges and
    per-chunk causal affine_selects (never touching unwritten PSUM).
  - AV in O-orientation: stationary = exp(S^T) chunk [128k, 128q], moving =
    V [128k, 65] (col 64 = ones -> softmax sums land in ov col 64). 65-row
    bf16 matmuls halve AV PE time vs the O^T orientation. PSUM zero-region
    note: start=True arms pending-zero for the whole 2KB region, so it is
    issued exactly once per head (first AV); later qq slices first-touch-
    replace and then accumulate.
  - Normalization: per-partition reciprocal + tensor_tensor multiply (queries
    are on partitions in O-layout), then O is transposed back to O^T via
    128-row matmuls against an identity for the output projection.
  - Output projection to partial y^T (+bo/2) in bf16; pairwise ReduceScatter
    (bf16 payload halves the 15us+bytes/40GBps collective cost; 4 t-slabs)
    sums partner partials; core even keeps c' 0:512, odd keeps 512:1024.
    The final slab's outproj pre-accumulates head-pairs 0..2 during the last
    head's slots so only the cl=3 matmul + one combine sit before the last
    (fully exposed) ReduceScatter.
Host reassembles the [B, T, C] f32 output by transposing/concatenating slabs.
"""

import numpy as np
import ml_dtypes

import concourse.bass as bass
import concourse.mybir as mybir
from concourse import bacc
from concourse.tile import TileContext
from concourse.bass_utils import run_bass_kernel_spmd

F32 = mybir.dt.float32
BF16 = mybir.dt.bfloat16

B, T, C = 4, 2048, 1024
H, D = 16, 64
HC = 8           # heads per core
NPAIR = HC // 2  # head pairs
CCn = C // 128   # 8 contraction chunks
TTn = T // 512   # 4 slabs of 512
JCn = T // 128   # 16 key chunks of 128
N_CORES = 8
RG = [[0, 1], [2, 3], [4, 5], [6, 7]]


def build_nc(with_rs: bool = True, debug: bool = False):
    nc = bacc.Bacc(None, target_bir_lowering=False)

    xT = nc.declare_dram_parameter("xT", [C, T], BF16, isOutput=False)
    wq = nc.declare_dram_parameter("wq", [C, 512], BF16, isOutput=False)
    wk = nc.declare_dram_parameter("wk", [C, 512], BF16, isOutput=False)
    wv = nc.declare_dram_parameter("wv", [C, 512], BF16, isOutput=False)
    wot = nc.declare_dram_parameter("wot", [512, C], BF16, isOutput=False)
    bo2 = nc.declare_dram_parameter("bo2", [128, 8], F32, isOutput=False)
    y = nc.declare_dram_parameter("y", [TTn, 512, 512], BF16, isOutput=True)
    if debug:
        dbg_kq = nc.declare_dram_parameter("dbg_kq", [2, 128, 512], BF16,
                                           isOutput=True)
        dbg_v = nc.declare_dram_parameter("dbg_v", [4, 128, 520], BF16,
                                          isOutput=True)
        dbg_pt = nc.declare_dram_parameter("dbg_pt", [2, 128, 1024], BF16,
                                           isOutput=True)
        dbg_ov = nc.declare_dram_parameter("dbg_ov", [128, 260], F32,
                                           isOutput=True)
        dbg_op = nc.declare_dram_parameter("dbg_op", [128, 512], BF16,
                                           isOutput=True)
        dbg_ot = nc.declare_dram_parameter("dbg_ot", [128, 512], BF16,
                                           isOutput=True)

    with TileContext(nc) as tc:
        with (
            tc.tile_pool(name="persist", bufs=1) as sb,
            tc.tile_pool(name="psum", bufs=1, space="PSUM") as psum,
            tc.tile_pool(name="dram", bufs=1, space="DRAM") as dram,
        ):
            # ---- persistent SBUF tiles (per-slab splits avoid false deps
            # between interleaved projection writes and attention reads) ----
            qt = {(p, s): sb.tile([128, 512], BF16, tag=f"qt{p}_{s}", name=f"qt{p}_{s}")
                  for p in range(NPAIR) for s in range(TTn)}
            kt = {(p, s): sb.tile([128, 512], BF16, tag=f"kt{p}_{s}", name=f"kt{p}_{s}")
                  for p in range(NPAIR) for s in range(TTn)}
            v = [sb.tile([128, 65 * HC], BF16, tag=f"v{j}", name=f"v{j}") for j in range(JCn)]
            ot = {(p, s): sb.tile([128, 512], BF16, tag=f"ot{p}_{s}", name=f"ot{p}_{s}")
                  for p in range(NPAIR) for s in range(TTn)}
            wqb = sb.tile([128, CCn * 512], BF16, tag="wqb", name="wqb")
            wkb = sb.tile([128, CCn * 512], BF16, tag="wkb", name="wkb")
            wvb = sb.tile([128, CCn * 512], BF16, tag="wvb", name="wvb")
            wob = sb.tile([128, 4 * C], BF16, tag="wob", name="wob")
            ones8 = sb.tile([128, HC], BF16, tag="ones8")
            ident = sb.tile([128, 128], BF16, tag="ident")
            bo_sb = sb.tile([128, 8], F32, tag="bo_sb")

            y_part = dram.tile([TTn, 1024, 512], BF16)
            rs_out = dram.tile([TTn, 512, 512], BF16)

            # ---- prologue DMAs: one batched strided transfer per tensor
            # (per-DMA queue overhead ~0.6us dominates small transfers) ----
            xtb_of = {}

            def issue_xts(s):
                i0 = s * 512
                t = sb.tile([128, CCn * 512], BF16, tag="xtb", bufs=2,
                            name=f"xtb{s}")
                nc.sync.dma_start(
                    out=t[:].rearrange("p (cc t) -> p cc t", cc=CCn),
                    in_=xT[:, i0:i0 + 512].rearrange(
                        "(cc p) t -> p cc t", cc=CCn))
                xtb_of[s] = t

            t0_ = sb.tile([128, CCn * 512], BF16, tag="xtb", bufs=2,
                          name="xtb0")
            xtb_of[0] = t0_
            for hh in range(4):
                cs = slice(hh * 2 * 512, (hh + 1) * 2 * 512)
                rs_ = slice(hh * 2 * 128, (hh + 1) * 2 * 128)
                nc.sync.dma_start(
                    out=wkb[:, cs].rearrange("p (cc j) -> p cc j", cc=2),
                    in_=wk[rs_, :].rearrange("(cc p) j -> p cc j", cc=2))
                nc.sync.dma_start(
                    out=t0_[:, cs].rearrange("p (cc t) -> p cc t", cc=2),
                    in_=xT[rs_, 0:512].rearrange("(cc p) t -> p cc t", cc=2))
                nc.sync.dma_start(
                    out=wqb[:, cs].rearrange("p (cc j) -> p cc j", cc=2),
                    in_=wq[rs_, :].rearrange("(cc p) j -> p cc j", cc=2))
            nc.sync.dma_start(
                out=wvb[:].rearrange("p (cc j) -> p cc j", cc=CCn),
                in_=wv[:].rearrange("(cc p) j -> p cc j", cc=CCn))
            nc.sync.dma_start(
                out=wob[:].rearrange("p (cl j) -> p cl j", cl=4),
                in_=wot[:].rearrange("(cl p) j -> p cl j", cl=4))
            nc.sync.dma_start(out=bo_sb[:], in_=bo2[:])
            nc.vector.memset(ones8[:], 1.0)
            # identity for O -> O^T transposes: memset 1, keep only the diagonal
            nc.vector.memset(ident[:], 1.0)
            nc.gpsimd.affine_select(
                out=ident[:], in_=ident[:], compare_op=mybir.AluOpType.is_ge,
                fill=0.0, base=0, pattern=[[1, 128]], channel_multiplier=-1)
            nc.gpsimd.affine_select(
                out=ident[:], in_=ident[:], compare_op=mybir.AluOpType.is_ge,
                fill=0.0, base=0, pattern=[[-1, 128]], channel_multiplier=1)

            # ---- projection emission units (filler work for the PE) ----
            # Each unit emits ~2-4 matmuls (~0.4us of PE). A group's PSUM tile
            # is allocated by its first unit; the last unit emits the copy out.
            def make_proj_units(s, defer_qk_pairs=()):
                units = []
                unit_chunks = []   # list of per-group unit lists, woven below
                deferred = []  # (unit, deadline_slot) for the NEXT slab
                xtb = xtb_of[s]

                def qk_group(wb, pair, dst_tile, defer_dl=None):
                    cell = {}
                    for cc0 in range(0, CCn, 2):
                        def u(cc0=cc0, wb=wb, pair=pair, cell=cell):
                            if cc0 == 0:
                                cell["ps"] = psum.tile([128, 512], F32,
                                                       tag="yps", bufs=2,
                                                       name="qkps")
                            ps = cell["ps"]
                            for cc in (cc0, cc0 + 1):
                                nc.tensor.matmul(
                                    ps[:],
                                    wb[:, cc * 512 + pair * 128:
                                       cc * 512 + pair * 128 + 128],
                                    xtb[:, cc * 512:(cc + 1) * 512],
                                    start=(cc == 0), stop=(cc == CCn - 1),
                                    skip_group_check=True)
                            if cc0 == CCn - 2:
                                nc.vector.tensor_copy(dst_tile[:], ps[:])
                        if defer_dl is None:
                            cur_chunk.append(u)
                        else:
                            deferred.append((u, defer_dl))

                ng = 2 * (s + 1)  # groups/head in the consuming slab
                for p in range(NPAIR):
                    cur_chunk = []
                    if p in defer_qk_pairs:
                        # consumed in the NEXT window, whose head list starts
                        # at h=2: head (s,2p) sits at slot ng*(2p-2)
                        dl = ng * (2 * p - 2) - 1
                        qk_group(wkb, p, kt[(p, s)], defer_dl=dl)
                        qk_group(wqb, p, qt[(p, s)], defer_dl=dl)
                    else:
                        qk_group(wkb, p, kt[(p, s)])
                        qk_group(wqb, p, qt[(p, s)])
                    unit_chunks.append(cur_chunk)

                def v_group(jc):
                    jl = jc * 128 - s * 512
                    cell = {}
                    for i in range(4):
                        def u(i=i, jc=jc, jl=jl, cell=cell):
                            if i == 0:
                                cell["ps"] = psum.tile([128, 512], F32,
                                                       tag="yps", bufs=2, name="vps")
                            ps = cell["ps"]
                            for m in range(4):
                                gm = i * 4 + m
                                g, cc = gm // 8, gm % 8
                                nc.tensor.matmul(
                                    ps[:, g * 256:(g + 1) * 256],
                                    xtb[:, cc * 512 + jl:cc * 512 + jl + 128],
                                    wvb[:, cc * 512 + g * 256:
                                        cc * 512 + g * 256 + 256],
                                    start=(cc == 0), stop=(cc == CCn - 1),
                                    skip_group_check=True)
                            if i == 3:
                                vv = v[jc][:].rearrange(
                                    "p (h e) -> p h e", h=HC, e=65)
                                nc.vector.tensor_copy(vv[:, :, 0:64], ps[:])
                                nc.vector.tensor_copy(vv[:, :, 64:65],
                                                      ones8[:])
                        cur_chunk.append(u)

                for jc in range(4 * s, 4 * s + 4):
                    cur_chunk = []
                    v_group(jc)
                    unit_chunks.append(cur_chunk)
                # weave: K0Q0, V0, V1, K1Q1, V2, V3, K2Q2, K3Q3 — V chunks
                # early enough that pulled-forward heads of slab s (processed
                # late in window s-1) see their v[] tiles written in time
                qks, vs = unit_chunks[:NPAIR], unit_chunks[NPAIR:]
                for i, chunk in enumerate([qks[0], vs[0], vs[1], qks[1],
                                           vs[2], vs[3], qks[2], qks[3]]):
                    units.extend(chunk)
                deferred.sort(key=lambda t: t[1])
                return units, deferred

            # interleave order inside PROJ(0) so tt0/h0 attention can start
            # as early as possible: K0,Q0,V0,V1 then the rest
            units0, _ = make_proj_units(0)
            # units0 layout: [K0(4), Q0(4), K1(4), Q1(4), ... V groups(4x4)]
            order0 = (units0[0:8] + units0[32:40] + units0[8:16]
                      + units0[40:48] + units0[16:32])
            for u in order0:
                u()
            deferred_next = []

            # ---- attention + interleaved projections ----
            pt_pool = sb
            held_q = []          # pending AV emissions (2-deep pipeline)
            pending_norm = None  # (tt, h, ov) awaiting recip+TT
            opair_cell = {}      # pair -> o_pair staging tile

            def emit_avs(hd):
                tt_, h_, pt_, a_of = hd
                ovt = ov_of[(tt_, h_)]
                for qq in range(4):
                    for idx, (jc, a) in enumerate(a_of):
                        if a <= qq * 128:
                            c0 = idx * 512 + qq * 128
                            # start only once per head: start=True arms a
                            # pending-zero over the whole 2KB PSUM zero
                            # region, so later qq slices first-touch-replace
                            # rather than re-arm (which would wipe earlier
                            # slices' partials on their next accumulate).
                            nc.tensor.matmul(
                                ovt[:, qq * 65:qq * 65 + 65],
                                pt_[:, c0:c0 + 128],
                                v[jc][:, h_ * 65:(h_ + 1) * 65],
                                start=(jc == 4 * tt_ and qq == 0),
                                stop=(jc == 4 * tt_ - 1) if tt_ >= 1
                                else (jc == qq),
                                skip_group_check=True)

            ov_of = {}

            def emit_norm(tt, h, ov):
                p, e = h // 2, h % 2
                if e == 0:
                    opair_cell[p] = sb.tile([128, 512], BF16, tag="opair",
                                            bufs=2, name=f"op{tt}{p}")
                opair = opair_cell[p]
                ovr = ov[:].rearrange("p (q o e) -> p q o e", q=4, o=1, e=65)
                rl = sb.tile([128, 4], F32, tag="rl", bufs=2, name="rl")
                nc.vector.reciprocal(
                    rl[:].rearrange("p (q o) -> p q o", q=4, o=1),
                    ov[:].rearrange("p (q e) -> p q e", q=4, e=65)[:, :, 64:65])
                opr = opair[:].rearrange("p (q hh e) -> p q hh e",
                                         q=4, hh=2, e=64)
                nc.vector.tensor_mul(
                    opr[:, :, e:e + 1, :], ovr[:, :, :, 0:64],
                    rl[:].rearrange("p (q o u) -> p q o u", q=4, o=1, u=1)
                    .broadcast_to((128, 4, 1, 64)))
                if debug and (tt, h) == (0, 0):
                    ov_stage = sb.tile([128, 260], F32, tag="dbgov",
                                       name="dbgov")
                    nc.vector.tensor_copy(ov_stage[:], ov[:])
                    nc.sync.dma_start(out=dbg_ov[:], in_=ov_stage[:])
                if e == 1:
                    # pair complete: transpose O -> O^T into ot[(p, tt)]
                    trp = psum.tile([128, 512], F32, tag="yps", bufs=2,
                                    name=f"tr{tt}{p}")
                    for qq in range(4):
                        nc.tensor.matmul(
                            trp[:, qq * 128:(qq + 1) * 128],
                            opair[:, qq * 128:(qq + 1) * 128],
                            ident[:], start=True, stop=True,
                            skip_group_check=True)
                    nc.vector.tensor_copy(ot[(p, tt)][:], trp[:])
                    if debug and (tt, h) == (0, 1):
                        nc.sync.dma_start(out=dbg_op[:], in_=opair[:])
                        nc.sync.dma_start(out=dbg_ot[:], in_=ot[(p, tt)][:])
                    del opair_cell[p]

            pg_tiles = {}

            def emit_partial_outproj(cp):
                # cl 0..2 partial sum for the final slab, staged to SBUF so
                # the epilogue only needs the cl=3 matmul + one combine
                yps = psum.tile([128, 512], F32, tag="yps", bufs=2,
                                name=f"pg{cp}")
                for cl in range(3):
                    nc.tensor.matmul(
                        yps[:], wob[:, cl * C + cp * 128:cl * C + cp * 128 + 128],
                        ot[(cl, TTn - 1)][:], start=(cl == 0), stop=(cl == 2),
                        skip_group_check=True)
                pg = sb.tile([128, 512], F32, tag=f"pg{cp}", name=f"pgs{cp}")
                nc.vector.tensor_copy(pg[:], yps[:])
                pg_tiles[cp] = pg

            def emit_outproj_group(tt, cp):
                yps = psum.tile([128, 512], F32, tag="yps", bufs=2,
                                name=f"yps{tt}{cp}")
                pg = pg_tiles.get(cp) if tt == TTn - 1 else None
                cl0 = 3 if pg is not None else 0
                for cl in range(cl0, 4):
                    nc.tensor.matmul(
                        yps[:], wob[:, cl * C + cp * 128:cl * C + cp * 128 + 128],
                        ot[(cl, tt)][:], start=(cl == cl0), stop=(cl == 3),
                        skip_group_check=True)
                ysb = sb.tile([128, 512], BF16, tag="ysb", bufs=8, name="ysb")
                if pg is not None:
                    nc.vector.scalar_tensor_tensor(
                        ysb[:], yps[:], bo_sb[:, cp:cp + 1], pg[:],
                        mybir.AluOpType.add, mybir.AluOpType.add)
                else:
                    nc.vector.tensor_scalar_add(ysb[:], yps[:],
                                                bo_sb[:, cp:cp + 1])
                nc.sync.dma_start(
                    out=y_part[tt, cp * 128:(cp + 1) * 128, :], in_=ysb[:])

            def emit_rs(tt):
                if with_rs:
                    nc.gpsimd.collective_compute(
                        "ReduceScatter", mybir.AluOpType.add,
                        replica_groups=RG,
                        ins=[y_part[tt]], outs=[rs_out[tt]])
                    nc.sync.dma_start(out=y[tt], in_=rs_out[tt])
                else:
                    nc.sync.dma_start(out=y[tt], in_=y_part[tt, 0:512, :])

            windows = [
                [(0, h) for h in range(HC)],
                [(1, h) for h in range(HC)] + [(2, 0), (2, 1)],
                [(2, h) for h in range(2, HC)] + [(3, 0), (3, 1)],
                [(3, h) for h in range(2, HC)],
            ]
            for w in range(TTn):
                head_list = windows[w]
                deferred = deferred_next  # deferred here from the prior make
                if w < TTn - 1:
                    issue_xts(w + 1)
                    proj_units, deferred_next = make_proj_units(
                        w + 1, defer_qk_pairs=(2, 3) if w == 2 else ())
                else:
                    proj_units = []
                    deferred_next = []
                slot_idx = [0]
                emitted = [0]
                total_slots = sum(2 * (t_ + 1) for t_, _ in head_list)
                total_def = len(deferred)

                def emit_filler():
                    si = slot_idx[0]
                    slot_idx[0] += 1
                    rem_slots = total_slots - si
                    if proj_units and rem_slots > 0:
                        n = -(-len(proj_units) // rem_slots)  # ceil
                        for _ in range(min(n, 8)):
                            if proj_units:
                                proj_units.pop(0)()
                    # deferred units: emit when due (deadline) or to keep
                    # proportional pace across the whole slab
                    target = (si + 1) * total_def // max(total_slots, 1)
                    while deferred and (deferred[0][1] <= si + 1
                                        or emitted[0] < target):
                        deferred.pop(0)[0]()
                        emitted[0] += 1

                for tt, h in head_list:
                    i0 = tt * 512
                    n_g = 2 * (tt + 1)
                    g_seq = [2 * tt, 2 * tt + 1] + list(range(2 * tt))
                    p, e = h // 2, h % 2
                    ov_of[(tt, h)] = psum.tile([128, 260], F32, tag="ovps",
                                               bufs=2, name=f"ov{tt}{h}")
                    for gi, g in enumerate(g_seq):
                        jc0, jc1 = 2 * g, 2 * g + 1
                        a0 = max(0, (jc0 - 4 * tt)) * 128
                        a1 = max(0, (jc1 - 4 * tt)) * 128
                        st = psum.tile([128, 1024], F32, tag="stps", bufs=2,
                                       name=f"st{tt}{h}{g}")
                        for k, (jc, a) in enumerate(((jc0, a0), (jc1, a1))):
                            nc.tensor.matmul(
                                st[:, k * 512 + a:(k + 1) * 512],
                                kt[(p, jc // 4)][e * 64:(e + 1) * 64,
                                                 (jc % 4) * 128:
                                                 (jc % 4) * 128 + 128],
                                qt[(p, tt)][e * 64:(e + 1) * 64, a:512],
                                start=True, stop=True, skip_group_check=True)
                        while len(held_q) >= 3:
                            emit_avs(held_q.pop(0))
                        if gi == 0 and tt >= 1 and h in (1, 2, 3, 4):
                            for g2 in range(2):
                                emit_outproj_group(tt - 1, 2 * (h - 1) + g2)
                            if h == 4:
                                emit_rs(tt - 1)
                        if gi == 1 and pending_norm is not None:
                            nt, nh, _ = pending_norm
                            while any(e[0] == nt and e[1] == nh
                                      for e in held_q):
                                emit_avs(held_q.pop(0))
                            emit_norm(*pending_norm)
                            pending_norm = None
                        if (tt == TTn - 1 and h == 7 and len(pg_tiles) < 8):
                            emit_partial_outproj(len(pg_tiles))
                        emit_filler()
                        pt = pt_pool.tile([128, 1024], BF16, tag="pt", bufs=8,
                                          name=f"pt{tt}{h}{g}")
                        if debug and tt == 0 and h == 0:
                            nc.vector.memset(pt[:], 0.0)
                        if jc0 >= 4 * tt:
                            # band group: exact per-chunk exp + causal zeroing
                            # (regions outside [k*512+a, (k+1)*512) are never
                            # read downstream, so they stay unwritten)
                            for k, a in ((0, a0), (1, a1)):
                                lo, hi = k * 512 + a, (k + 1) * 512
                                nc.scalar.activation(
                                    pt[:, lo:hi], st[:, lo:hi],
                                    mybir.ActivationFunctionType.Exp)
                                nc.gpsimd.affine_select(
                                    out=pt[:, lo:hi], in_=pt[:, lo:hi],
                                    compare_op=mybir.AluOpType.is_ge,
                                    fill=0.0, base=0,
                                    pattern=[[1, 512 - a]],
                                    channel_multiplier=-1)
                        else:
                            nc.scalar.activation(
                                pt[:, 0:1024], st[:, 0:1024],
                                mybir.ActivationFunctionType.Exp)
                        if debug and tt == 0 and h == 0:
                            nc.sync.dma_start(out=dbg_pt[g], in_=pt[:])
                            if g == 0:
                                nc.sync.dma_start(out=dbg_kq[0],
                                                  in_=kt[(0, 0)][:])
                                nc.sync.dma_start(out=dbg_kq[1],
                                                  in_=qt[(0, 0)][:])
                                for _j in range(4):
                                    nc.sync.dma_start(out=dbg_v[_j],
                                                      in_=v[_j][:])
                        held_q.append((tt, h, pt, ((jc0, a0), (jc1, a1))))
                    if pending_norm is not None:  # tt0: only 2 groups per head
                        emit_norm(*pending_norm)
                        pending_norm = None
                    pending_norm = (tt, h, ov_of[(tt, h)])
                while proj_units:
                    proj_units.pop(0)()
                while deferred:
                    deferred.pop(0)[0]()
                if w == TTn - 1:
                    while held_q:
                        emit_avs(held_q.pop(0))
                    if pending_norm is not None:
                        emit_norm(*pending_norm)
                        pending_norm = None
                    for cp in range(8):
                        emit_outproj_group(TTn - 1, cp)
                    emit_rs(TTn - 1)

    nc.compile()
    return nc


_NC_CACHE = {}


def _get_nc(with_rs: bool = True):
    key = bool(with_rs)
    if key not in _NC_CACHE:
        _NC_CACHE[key] = build_nc(with_rs)
    return _NC_CACHE[key]


def make_in_maps(x, Wq, Wk, Wv, Wo, bo):
    bf16 = ml_dtypes.bfloat16
    x = np.asarray(x, dtype=np.float32)
    Wq = np.asarray(Wq, dtype=np.float32)
    Wk = np.asarray(Wk, dtype=np.float32)
    Wv = np.asarray(Wv, dtype=np.float32)
    Wo = np.asarray(Wo, dtype=np.float32)
    bo = np.asarray(bo, dtype=np.float32)

    scale = np.float32(C) ** np.float32(-0.5)
    in_maps = []
    for c in range(N_CORES):
        b, hoff = c // 2, (c % 2) * HC
        heads = slice(hoff, hoff + HC)
        xT_c = np.ascontiguousarray(x[b].T).astype(bf16)             # [C, T]
        wq_c = np.ascontiguousarray(
            np.concatenate(list(Wq[heads] * scale), axis=1)).astype(bf16)
        wk_c = np.ascontiguousarray(
            np.concatenate(list(Wk[heads]), axis=1)).astype(bf16)
        wv_c = np.ascontiguousarray(
            np.concatenate(list(Wv[heads]), axis=1)).astype(bf16)
        wot_c = np.ascontiguousarray(
            Wo[:, hoff * D:(hoff + HC) * D].T).astype(bf16)          # [512, C]
        bo2_c = np.ascontiguousarray((bo / 2.0).reshape(8, 128).T)   # [128, 8]
        in_maps.append({
            "xT": xT_c, "wq": wq_c, "wk": wk_c, "wv": wv_c,
            "wot": wot_c, "bo2": bo2_c,
        })
    return in_maps


def kernel(x, Wq, Wk, Wv, Wo, bo):
    nc = _get_nc(with_rs=True)
    in_maps = make_in_maps(x, Wq, Wk, Wv, Wo, bo)
    # The axon-tunneled devices occasionally fail transiently
    # (NRT_EXEC_UNIT_UNRECOVERABLE / tunnel hangup); a retry recovers.
    last_err = None
    for _ in range(3):
        try:
            res = run_bass_kernel_spmd(nc, in_maps, list(range(N_CORES))).results
            break
        except Exception as e:  # noqa: BLE001
            last_err = e
            import time
            time.sleep(5)
    else:
        raise last_err

    out = np.empty((B, T, C), dtype=np.float32)
    for c in range(N_CORES):
        b, e = c // 2, c % 2
        yc = np.asarray(res[c]["y"], dtype=np.float32)  # [tt, c' slab, t]
        for tt in range(TTn):
            out[b, tt * 512:(tt + 1) * 512, e * 512:(e + 1) * 512] = yc[tt].T
    return out


# revision 53
# speedup vs baseline: 1.0937x; 1.0013x over previous
"""Multi-head causal attention (B=4, T=2048, C=1024, H=16, D=64) on 8 trn2 cores.

Sharding: tensor-parallel over heads within batch core-pairs.
  core c -> batch b = c//2, heads hoff..hoff+7 where hoff = (c%2)*8.

v2 design (all-bf16 dataflow; 377.8us baseline -> 275.1us):
  - Projections (Q^T/K^T per head-pair, V per key-chunk with a folded ones
    column) are software-pipelined INTO the attention loop as PE filler work:
    slab s+1's projection matmuls are paced between slab s's attention
    groups, because attention is ACT(exp)-bound while projections are pure
    PE. Inputs arrive as a handful of large batched strided DMAs (per-DMA
    queue overhead ~0.6us dominates small transfers).
  - Attention emission is organized in 4 windows whose head lists pull the
    first two heads of the next slab forward (their exps fill the ACT-idle
    tail of the previous window); within a head, the diagonal band groups
    run FIRST and off-band groups last, so the next head's scores never
    stall on a bunched-up chain of band exps.
  - Scores per head in S^T = [key, query] orientation, exp without
    max-subtraction (scores ~N(0, 0.25^2)), exact per-chunk exp ranges and
    per-chunk causal affine_selects (never touching unwritten PSUM).
  - AV in O-orientation: stationary = exp(S^T) chunk [128k, 128q], moving =
    V [128k, 65] (col 64 = ones -> softmax sums land in ov col 64). 65-row
    bf16 matmuls halve AV PE time vs the O^T orientation. AVs are emitted
    through a 3-deep queue (three groups late), so they never stall on the
    exp/select chain of their own group. PSUM zero-region
    note: start=True arms pending-zero for the whole 2KB region, so it is
    issued exactly once per head (first AV); later qq slices first-touch-
    replace and then accumulate.
  - Normalization: per-partition reciprocal + tensor_tensor multiply (queries
    are on partitions in O-layout), then O is transposed back to O^T via
    128-row matmuls against an identity for the output projection.
  - Output projection to partial y^T (+bo/2) in bf16; pairwise ReduceScatter
    (bf16 payload halves the 15us+bytes/40GBps collective cost; 4 t-slabs)
    sums partner partials; core even keeps c' 0:512, odd keeps 512:1024.
    The final slab's outproj pre-accumulates head-pairs 0..2 during the last
    head's slots so only the cl=3 matmul + one combine sit before the last
    (fully exposed) ReduceScatter.
Host reassembles the [B, T, C] f32 output by transposing/concatenating slabs.
"""

import numpy as np
import ml_dtypes

import concourse.bass as bass
import concourse.mybir as mybir
from concourse import bacc
from concourse.tile import TileContext
from concourse.bass_utils import run_bass_kernel_spmd

F32 = mybir.dt.float32
BF16 = mybir.dt.bfloat16

B, T, C = 4, 2048, 1024
H, D = 16, 64
HC = 8           # heads per core
NPAIR = HC // 2  # head pairs
CCn = C // 128   # 8 contraction chunks
TTn = T // 512   # 4 slabs of 512
JCn = T // 128   # 16 key chunks of 128
N_CORES = 8
RG = [[0, 1], [2, 3], [4, 5], [6, 7]]


def build_nc(with_rs: bool = True, debug: bool = False):
    nc = bacc.Bacc(None, target_bir_lowering=False)

    xT = nc.declare_dram_parameter("xT", [C, T], BF16, isOutput=False)
    wq = nc.declare_dram_parameter("wq", [C, 512], BF16, isOutput=False)
    wk = nc.declare_dram_parameter("wk", [C, 512], BF16, isOutput=False)
    wv = nc.declare_dram_parameter("wv", [C, 512], BF16, isOutput=False)
    wot = nc.declare_dram_parameter("wot", [512, C], BF16, isOutput=False)
    bo2 = nc.declare_dram_parameter("bo2", [128, 8], F32, isOutput=False)
    y = nc.declare_dram_parameter("y", [TTn, 512, 512], BF16, isOutput=True)
    if debug:
        dbg_kq = nc.declare_dram_parameter("dbg_kq", [2, 128, 512], BF16,
                                           isOutput=True)
        dbg_v = nc.declare_dram_parameter("dbg_v", [4, 128, 520], BF16,
                                          isOutput=True)
        dbg_pt = nc.declare_dram_parameter("dbg_pt", [2, 128, 1024], BF16,
                                           isOutput=True)
        dbg_ov = nc.declare_dram_parameter("dbg_ov", [128, 260], F32,
                                           isOutput=True)
        dbg_op = nc.declare_dram_parameter("dbg_op", [128, 512], BF16,
                                           isOutput=True)
        dbg_ot = nc.declare_dram_parameter("dbg_ot", [128, 512], BF16,
                                           isOutput=True)

    with TileContext(nc) as tc:
        with (
            tc.tile_pool(name="persist", bufs=1) as sb,
            tc.tile_pool(name="psum", bufs=1, space="PSUM") as psum,
            tc.tile_pool(name="dram", bufs=1, space="DRAM") as dram,
        ):
            # ---- persistent SBUF tiles (per-slab splits avoid false deps
            # between interleaved projection writes and attention reads) ----
            qt = {(p, s): sb.tile([128, 512], BF16, tag=f"qt{p}_{s}", name=f"qt{p}_{s}")
                  for p in range(NPAIR) for s in range(TTn)}
            kt = {(p, s): sb.tile([128, 512], BF16, tag=f"kt{p}_{s}", name=f"kt{p}_{s}")
                  for p in range(NPAIR) for s in range(TTn)}
            v = [sb.tile([128, 65 * HC], BF16, tag=f"v{j}", name=f"v{j}") for j in range(JCn)]
            ot = {(p, s): sb.tile([128, 512], BF16, tag=f"ot{p}_{s}", name=f"ot{p}_{s}")
                  for p in range(NPAIR) for s in range(TTn)}
            wqb = sb.tile([128, CCn * 512], BF16, tag="wqb", name="wqb")
            wkb = sb.tile([128, CCn * 512], BF16, tag="wkb", name="wkb")
            wvb = sb.tile([128, CCn * 512], BF16, tag="wvb", name="wvb")
            wob = sb.tile([128, 4 * C], BF16, tag="wob", name="wob")
            ones8 = sb.tile([128, HC], BF16, tag="ones8")
            ident = sb.tile([128, 128], BF16, tag="ident")
            bo_sb = sb.tile([128, 8], F32, tag="bo_sb")

            y_part = dram.tile([TTn, 1024, 512], BF16)
            rs_out = dram.tile([TTn, 512, 512], BF16)

            # ---- prologue DMAs: one batched strided transfer per tensor
            # (per-DMA queue overhead ~0.6us dominates small transfers) ----
            xtb_of = {}

            def issue_xts(s):
                i0 = s * 512
                t = sb.tile([128, CCn * 512], BF16, tag="xtb", bufs=2,
                            name=f"xtb{s}")
                nc.sync.dma_start(
                    out=t[:].rearrange("p (cc t) -> p cc t", cc=CCn),
                    in_=xT[:, i0:i0 + 512].rearrange(
                        "(cc p) t -> p cc t", cc=CCn))
                xtb_of[s] = t

            t0_ = sb.tile([128, CCn * 512], BF16, tag="xtb", bufs=2,
                          name="xtb0")
            xtb_of[0] = t0_
            for hh in range(4):
                cs = slice(hh * 2 * 512, (hh + 1) * 2 * 512)
                rs_ = slice(hh * 2 * 128, (hh + 1) * 2 * 128)
                nc.sync.dma_start(
                    out=wkb[:, cs].rearrange("p (cc j) -> p cc j", cc=2),
                    in_=wk[rs_, :].rearrange("(cc p) j -> p cc j", cc=2))
                nc.sync.dma_start(
                    out=t0_[:, cs].rearrange("p (cc t) -> p cc t", cc=2),
                    in_=xT[rs_, 0:512].rearrange("(cc p) t -> p cc t", cc=2))
                nc.sync.dma_start(
                    out=wqb[:, cs].rearrange("p (cc j) -> p cc j", cc=2),
                    in_=wq[rs_, :].rearrange("(cc p) j -> p cc j", cc=2))
            nc.sync.dma_start(
                out=wvb[:].rearrange("p (cc j) -> p cc j", cc=CCn),
                in_=wv[:].rearrange("(cc p) j -> p cc j", cc=CCn))
            nc.sync.dma_start(
                out=wob[:].rearrange("p (cl j) -> p cl j", cl=4),
                in_=wot[:].rearrange("(cl p) j -> p cl j", cl=4))
            nc.sync.dma_start(out=bo_sb[:], in_=bo2[:])
            nc.vector.memset(ones8[:], 1.0)
            # identity for O -> O^T transposes: memset 1, keep only the diagonal
            nc.vector.memset(ident[:], 1.0)
            nc.gpsimd.affine_select(
                out=ident[:], in_=ident[:], compare_op=mybir.AluOpType.is_ge,
                fill=0.0, base=0, pattern=[[1, 128]], channel_multiplier=-1)
            nc.gpsimd.affine_select(
                out=ident[:], in_=ident[:], compare_op=mybir.AluOpType.is_ge,
                fill=0.0, base=0, pattern=[[-1, 128]], channel_multiplier=1)

            # ---- projection emission units (filler work for the PE) ----
            # Each unit emits ~2-4 matmuls (~0.4us of PE). A group's PSUM tile
            # is allocated by its first unit; the last unit emits the copy out.
            def make_proj_units(s, defer_qk_pairs=()):
                units = []
                unit_chunks = []   # list of per-group unit lists, woven below
                deferred = []  # (unit, deadline_slot) for the NEXT slab
                xtb = xtb_of[s]

                def qk_group(wb, pair, dst_tile, defer_dl=None):
                    cell = {}
                    for cc0 in range(0, CCn, 2):
                        def u(cc0=cc0, wb=wb, pair=pair, cell=cell):
                            if cc0 == 0:
                                cell["ps"] = psum.tile([128, 512], F32,
                                                       tag="yps", bufs=2,
                                                       name="qkps")
                            ps = cell["ps"]
                            for cc in (cc0, cc0 + 1):
                                nc.tensor.matmul(
                                    ps[:],
                                    wb[:, cc * 512 + pair * 128:
                                       cc * 512 + pair * 128 + 128],
                                    xtb[:, cc * 512:(cc + 1) * 512],
                                    start=(cc == 0), stop=(cc == CCn - 1),
                                    skip_group_check=True)
                            if cc0 == CCn - 2:
                                nc.vector.tensor_copy(dst_tile[:], ps[:])
                        if defer_dl is None:
                            cur_chunk.append(u)
                        else:
                            deferred.append((u, defer_dl))

                ng = 2 * (s + 1)  # groups/head in the consuming slab
                for p in range(NPAIR):
                    cur_chunk = []
                    if p in defer_qk_pairs:
                        # consumed in the NEXT window, whose head list starts
                        # at h=2: head (s,2p) sits at slot ng*(2p-2)
                        dl = ng * (2 * p - 2) - 1
                        qk_group(wkb, p, kt[(p, s)], defer_dl=dl)
                        qk_group(wqb, p, qt[(p, s)], defer_dl=dl)
                    else:
                        qk_group(wkb, p, kt[(p, s)])
                        qk_group(wqb, p, qt[(p, s)])
                    unit_chunks.append(cur_chunk)

                def v_group(jc):
                    jl = jc * 128 - s * 512
                    cell = {}
                    for i in range(4):
                        def u(i=i, jc=jc, jl=jl, cell=cell):
                            if i == 0:
                                cell["ps"] = psum.tile([128, 512], F32,
                                                       tag="yps", bufs=2, name="vps")
                            ps = cell["ps"]
                            for m in range(4):
                                gm = i * 4 + m
                                g, cc = gm // 8, gm % 8
                                nc.tensor.matmul(
                                    ps[:, g * 256:(g + 1) * 256],
                                    xtb[:, cc * 512 + jl:cc * 512 + jl + 128],
                                    wvb[:, cc * 512 + g * 256:
                                        cc * 512 + g * 256 + 256],
                                    start=(cc == 0), stop=(cc == CCn - 1),
                                    skip_group_check=True)
                            if i == 3:
                                vv = v[jc][:].rearrange(
                                    "p (h e) -> p h e", h=HC, e=65)
                                nc.vector.tensor_copy(vv[:, :, 0:64], ps[:])
                                nc.vector.tensor_copy(vv[:, :, 64:65],
                                                      ones8[:])
                        cur_chunk.append(u)

                for jc in range(4 * s, 4 * s + 4):
                    cur_chunk = []
                    v_group(jc)
                    unit_chunks.append(cur_chunk)
                # weave: K0Q0, V0, V1, K1Q1, V2, V3, K2Q2, K3Q3 — V chunks
                # early enough that pulled-forward heads of slab s (processed
                # late in window s-1) see their v[] tiles written in time
                qks, vs = unit_chunks[:NPAIR], unit_chunks[NPAIR:]
                for i, chunk in enumerate([qks[0], vs[0], vs[1], qks[1],
                                           vs[2], vs[3], qks[2], qks[3]]):
                    units.extend(chunk)
                deferred.sort(key=lambda t: t[1])
                return units, deferred

            # interleave order inside PROJ(0) so tt0/h0 attention can start
            # as early as possible: K0,Q0,V0,V1 then the rest
            units0, _ = make_proj_units(0)
            # units0 layout: [K0(4), Q0(4), K1(4), Q1(4), ... V groups(4x4)]
            order0 = (units0[0:8] + units0[32:40] + units0[8:16]
                      + units0[40:48] + units0[16:32])
            for u in order0:
                u()
            deferred_next = []

            # ---- attention + interleaved projections ----
            pt_pool = sb
            held_q = []          # pending AV emissions (2-deep pipeline)
            pending_norm = None  # (tt, h, ov) awaiting recip+TT
            opair_cell = {}      # pair -> o_pair staging tile

            def emit_avs(hd):
                tt_, h_, pt_, a_of = hd
                ovt = ov_of[(tt_, h_)]
                for qq in range(4):
                    for idx, (jc, a) in enumerate(a_of):
                        if a <= qq * 128:
                            c0 = idx * 512 + qq * 128
                            # start only once per head: start=True arms a
                            # pending-zero over the whole 2KB PSUM zero
                            # region, so later qq slices first-touch-replace
                            # rather than re-arm (which would wipe earlier
                            # slices' partials on their next accumulate).
                            nc.tensor.matmul(
                                ovt[:, qq * 65:qq * 65 + 65],
                                pt_[:, c0:c0 + 128],
                                v[jc][:, h_ * 65:(h_ + 1) * 65],
                                start=(jc == 4 * tt_ and qq == 0),
                                stop=(jc == 4 * tt_ - 1) if tt_ >= 1
                                else (jc == qq),
                                skip_group_check=True)

            ov_of = {}

            def emit_norm(tt, h, ov):
                p, e = h // 2, h % 2
                if e == 0:
                    opair_cell[p] = sb.tile([128, 512], BF16, tag="opair",
                                            bufs=2, name=f"op{tt}{p}")
                opair = opair_cell[p]
                ovr = ov[:].rearrange("p (q o e) -> p q o e", q=4, o=1, e=65)
                rl = sb.tile([128, 4], F32, tag="rl", bufs=2, name="rl")
                nc.vector.reciprocal(
                    rl[:].rearrange("p (q o) -> p q o", q=4, o=1),
                    ov[:].rearrange("p (q e) -> p q e", q=4, e=65)[:, :, 64:65])
                opr = opair[:].rearrange("p (q hh e) -> p q hh e",
                                         q=4, hh=2, e=64)
                nc.vector.tensor_mul(
                    opr[:, :, e:e + 1, :], ovr[:, :, :, 0:64],
                    rl[:].rearrange("p (q o u) -> p q o u", q=4, o=1, u=1)
                    .broadcast_to((128, 4, 1, 64)))
                if debug and (tt, h) == (0, 0):
                    ov_stage = sb.tile([128, 260], F32, tag="dbgov",
                                       name="dbgov")
                    nc.vector.tensor_copy(ov_stage[:], ov[:])
                    nc.sync.dma_start(out=dbg_ov[:], in_=ov_stage[:])
                if e == 1:
                    # pair complete: transpose O -> O^T into ot[(p, tt)]
                    trp = psum.tile([128, 512], F32, tag="yps", bufs=2,
                                    name=f"tr{tt}{p}")
                    for qq in range(4):
                        nc.tensor.matmul(
                            trp[:, qq * 128:(qq + 1) * 128],
                            opair[:, qq * 128:(qq + 1) * 128],
                            ident[:], start=True, stop=True,
                            skip_group_check=True)
                    nc.vector.tensor_copy(ot[(p, tt)][:], trp[:])
                    if debug and (tt, h) == (0, 1):
                        nc.sync.dma_start(out=dbg_op[:], in_=opair[:])
                        nc.sync.dma_start(out=dbg_ot[:], in_=ot[(p, tt)][:])
                    del opair_cell[p]

            pg_tiles = {}

            def emit_partial_outproj(cp):
                # cl 0..2 partial sum for the final slab, staged to SBUF so
                # the epilogue only needs the cl=3 matmul + one combine
                yps = psum.tile([128, 512], F32, tag="yps", bufs=2,
                                name=f"pg{cp}")
                for cl in range(3):
                    nc.tensor.matmul(
                        yps[:], wob[:, cl * C + cp * 128:cl * C + cp * 128 + 128],
                        ot[(cl, TTn - 1)][:], start=(cl == 0), stop=(cl == 2),
                        skip_group_check=True)
                pg = sb.tile([128, 512], F32, tag=f"pg{cp}", name=f"pgs{cp}")
                nc.vector.tensor_copy(pg[:], yps[:])
                pg_tiles[cp] = pg

            def emit_outproj_group(tt, cp):
                yps = psum.tile([128, 512], F32, tag="yps", bufs=2,
                                name=f"yps{tt}{cp}")
                pg = pg_tiles.get(cp) if tt == TTn - 1 else None
                cl0 = 3 if pg is not None else 0
                for cl in range(cl0, 4):
                    nc.tensor.matmul(
                        yps[:], wob[:, cl * C + cp * 128:cl * C + cp * 128 + 128],
                        ot[(cl, tt)][:], start=(cl == cl0), stop=(cl == 3),
                        skip_group_check=True)
                ysb = sb.tile([128, 512], BF16, tag="ysb", bufs=8, name="ysb")
                if pg is not None:
                    nc.vector.scalar_tensor_tensor(
                        ysb[:], yps[:], bo_sb[:, cp:cp + 1], pg[:],
                        mybir.AluOpType.add, mybir.AluOpType.add)
                else:
                    nc.vector.tensor_scalar_add(ysb[:], yps[:],
                                                bo_sb[:, cp:cp + 1])
                nc.sync.dma_start(
                    out=y_part[tt, cp * 128:(cp + 1) * 128, :], in_=ysb[:])

            def emit_rs(tt):
                if with_rs:
                    nc.gpsimd.collective_compute(
                        "ReduceScatter", mybir.AluOpType.add,
                        replica_groups=RG,
                        ins=[y_part[tt]], outs=[rs_out[tt]])
                    nc.sync.dma_start(out=y[tt], in_=rs_out[tt])
                else:
                    nc.sync.dma_start(out=y[tt], in_=y_part[tt, 0:512, :])

            windows = [
                [(0, h) for h in range(HC)],
                [(1, h) for h in range(HC)] + [(2, 0), (2, 1)],
                [(2, h) for h in range(2, HC)] + [(3, 0), (3, 1)],
                [(3, h) for h in range(2, HC)],
            ]
            for w in range(TTn):
                head_list = windows[w]
                deferred = deferred_next  # deferred here from the prior make
                if w < TTn - 1:
                    issue_xts(w + 1)
                    proj_units, deferred_next = make_proj_units(
                        w + 1, defer_qk_pairs=(2, 3) if w == 2 else ())
                else:
                    proj_units = []
                    deferred_next = []
                slot_idx = [0]
                emitted = [0]
                total_slots = sum(2 * (t_ + 1) for t_, _ in head_list)
                total_def = len(deferred)

                def emit_filler():
                    si = slot_idx[0]
                    slot_idx[0] += 1
                    rem_slots = total_slots - si
                    if proj_units and rem_slots > 0:
                        n = -(-len(proj_units) // rem_slots)  # ceil
                        for _ in range(min(n, 8)):
                            if proj_units:
                                proj_units.pop(0)()
                    # deferred units: emit when due (deadline) or to keep
                    # proportional pace across the whole slab
                    target = (si + 1) * total_def // max(total_slots, 1)
                    while deferred and (deferred[0][1] <= si + 1
                                        or emitted[0] < target):
                        deferred.pop(0)[0]()
                        emitted[0] += 1

                for tt, h in head_list:
                    i0 = tt * 512
                    n_g = 2 * (tt + 1)
                    g_seq = [2 * tt, 2 * tt + 1] + list(range(2 * tt))
                    p, e = h // 2, h % 2
                    ov_of[(tt, h)] = psum.tile([128, 260], F32, tag="ovps",
                                               bufs=2, name=f"ov{tt}{h}")
                    for gi, g in enumerate(g_seq):
                        jc0, jc1 = 2 * g, 2 * g + 1
                        a0 = max(0, (jc0 - 4 * tt)) * 128
                        a1 = max(0, (jc1 - 4 * tt)) * 128
                        st = psum.tile([128, 1024], F32, tag="stps", bufs=2,
                                       name=f"st{tt}{h}{g}")
                        for k, (jc, a) in enumerate(((jc0, a0), (jc1, a1))):
                            nc.tensor.matmul(
                                st[:, k * 512 + a:(k + 1) * 512],
                                kt[(p, jc // 4)][e * 64:(e + 1) * 64,
                                                 (jc % 4) * 128:
                                                 (jc % 4) * 128 + 128],
                                qt[(p, tt)][e * 64:(e + 1) * 64, a:512],
                                start=True, stop=True, skip_group_check=True)
                        while len(held_q) >= 3:
                            emit_avs(held_q.pop(0))
                        if gi == 0 and tt >= 1 and h in (2, 3, 4, 5):
                            for g2 in range(2):
                                emit_outproj_group(tt - 1, 2 * (h - 2) + g2)
                            if h == 5:
                                emit_rs(tt - 1)
                        if gi == 1 and pending_norm is not None:
                            nt, nh, _ = pending_norm
                            while any(e[0] == nt and e[1] == nh
                                      for e in held_q):
                                emit_avs(held_q.pop(0))
                            emit_norm(*pending_norm)
                            pending_norm = None
                        if (tt == TTn - 1 and h == 7 and len(pg_tiles) < 8):
                            emit_partial_outproj(len(pg_tiles))
                        emit_filler()
                        pt = pt_pool.tile([128, 1024], BF16, tag="pt", bufs=8,
                                          name=f"pt{tt}{h}{g}")
                        if debug and tt == 0 and h == 0:
                            nc.vector.memset(pt[:], 0.0)
                        if jc0 >= 4 * tt:
                            # band group: exact per-chunk exp + causal zeroing
                            # (regions outside [k*512+a, (k+1)*512) are never
                            # read downstream, so they stay unwritten)
                            for k, a in ((0, a0), (1, a1)):
                                lo, hi = k * 512 + a, (k + 1) * 512
                                nc.scalar.activation(
                                    pt[:, lo:hi], st[:, lo:hi],
                                    mybir.ActivationFunctionType.Exp)
                                nc.gpsimd.affine_select(
                                    out=pt[:, lo:hi], in_=pt[:, lo:hi],
                                    compare_op=mybir.AluOpType.is_ge,
                                    fill=0.0, base=0,
                                    pattern=[[1, 512 - a]],
                                    channel_multiplier=-1)
                        else:
                            nc.scalar.activation(
                                pt[:, 0:1024], st[:, 0:1024],
                                mybir.ActivationFunctionType.Exp)
                        if debug and tt == 0 and h == 0:
                            nc.sync.dma_start(out=dbg_pt[g], in_=pt[:])
                            if g == 0:
                                nc.sync.dma_start(out=dbg_kq[0],
                                                  in_=kt[(0, 0)][:])
                                nc.sync.dma_start(out=dbg_kq[1],
                                                  in_=qt[(0, 0)][:])
                                for _j in range(4):
                                    nc.sync.dma_start(out=dbg_v[_j],
                                                      in_=v[_j][:])
                        held_q.append((tt, h, pt, ((jc0, a0), (jc1, a1))))
                    if pending_norm is not None:  # tt0: only 2 groups per head
                        emit_norm(*pending_norm)
                        pending_norm = None
                    pending_norm = (tt, h, ov_of[(tt, h)])
                while proj_units:
                    proj_units.pop(0)()
                while deferred:
                    deferred.pop(0)[0]()
                if w == TTn - 1:
                    while held_q:
                        emit_avs(held_q.pop(0))
                    if pending_norm is not None:
                        emit_norm(*pending_norm)
                        pending_norm = None
                    for cp in range(8):
                        emit_outproj_group(TTn - 1, cp)
                    emit_rs(TTn - 1)

    nc.compile()
    return nc


_NC_CACHE = {}


def _get_nc(with_rs: bool = True):
    key = bool(with_rs)
    if key not in _NC_CACHE:
        _NC_CACHE[key] = build_nc(with_rs)
    return _NC_CACHE[key]


def make_in_maps(x, Wq, Wk, Wv, Wo, bo):
    bf16 = ml_dtypes.bfloat16
    x = np.asarray(x, dtype=np.float32)
    Wq = np.asarray(Wq, dtype=np.float32)
    Wk = np.asarray(Wk, dtype=np.float32)
    Wv = np.asarray(Wv, dtype=np.float32)
    Wo = np.asarray(Wo, dtype=np.float32)
    bo = np.asarray(bo, dtype=np.float32)

    scale = np.float32(C) ** np.float32(-0.5)
    in_maps = []
    for c in range(N_CORES):
        b, hoff = c // 2, (c % 2) * HC
        heads = slice(hoff, hoff + HC)
        xT_c = np.ascontiguousarray(x[b].T).astype(bf16)             # [C, T]
        wq_c = np.ascontiguousarray(
            np.concatenate(list(Wq[heads] * scale), axis=1)).astype(bf16)
        wk_c = np.ascontiguousarray(
            np.concatenate(list(Wk[heads]), axis=1)).astype(bf16)
        wv_c = np.ascontiguousarray(
            np.concatenate(list(Wv[heads]), axis=1)).astype(bf16)
        wot_c = np.ascontiguousarray(
            Wo[:, hoff * D:(hoff + HC) * D].T).astype(bf16)          # [512, C]
        bo2_c = np.ascontiguousarray((bo / 2.0).reshape(8, 128).T)   # [128, 8]
        in_maps.append({
            "xT": xT_c, "wq": wq_c, "wk": wk_c, "wv": wv_c,
            "wot": wot_c, "bo2": bo2_c,
        })
    return in_maps


def kernel(x, Wq, Wk, Wv, Wo, bo):
    nc = _get_nc(with_rs=True)
    in_maps = make_in_maps(x, Wq, Wk, Wv, Wo, bo)
    # The axon-tunneled devices occasionally fail transiently
    # (NRT_EXEC_UNIT_UNRECOVERABLE / tunnel hangup); a retry recovers.
    last_err = None
    for _ in range(3):
        try:
            res = run_bass_kernel_spmd(nc, in_maps, list(range(N_CORES))).results
            break
        except Exception as e:  # noqa: BLE001
            last_err = e
            import time
            time.sleep(5)
    else:
        raise last_err

    out = np.empty((B, T, C), dtype=np.float32)
    for c in range(N_CORES):
        b, e = c // 2, c % 2
        yc = np.asarray(res[c]["y"], dtype=np.float32)  # [tt, c' slab, t]
        for tt in range(TTn):
            out[b, tt * 512:(tt + 1) * 512, e * 512:(e + 1) * 512] = yc[tt].T
    return out
